# revision 3
# baseline (speedup 1.0000x reference)
"""Trainium2 Bass kernel for the CPCA auxiliary loss (nn_CPCA_51754355917033).

Strategy (data-parallel over the env/batch dim n, 16 envs per core):
  - GRU runs fully in bf16.  The input-side gate terms (x@W_ih.T + biases)
    are folded into the same PSUM accumulation as W_hh@h via one-hot action
    matmuls against a 19-row table (emb@W_ih.T + b_ih [+ b_hh]), so the only
    per-step vector work is the g-gate tail; r/z evict straight from PSUM
    through the scalar engine's sigmoid.  The gate tail is split into four
    128-row H-chunks so the next step's matmuls (which contract one H-chunk
    each) start as soon as the first chunk of h is ready.
  - MLP: preds@W1a + b1 is hoisted out of the 21-block loop (computed once,
    stashed as bf16 "p1e"); per block only x@W1b runs on the tensor engine
    and the eviction adds p1e (vector) + ReLU (scalar/vector split).
  - Host combines the 8 cores' (pos_sum, neg_sum, denom) partials into the
    final scalar (the all-reduce of the sharding hint, done at unshard time).
"""

import numpy as np
import ml_dtypes

import concourse.bass as bass
import concourse.mybir as mybir
import concourse.tile as tile
from concourse import bacc
from concourse import bass_utils

BF16 = ml_dtypes.bfloat16
F8 = ml_dtypes.float8_e4m3
DT = mybir.dt
AF = mybir.ActivationFunctionType
ALU = mybir.AluOpType

N, T, H, K, S, F, EMB, NLOG, NEG = 128, 512, 512, 16, 16, 4, 32, 18, 20
COEFF = 0.1
NC = 8
NPC = N // NC          # envs per core
R = NPC * S            # GRU rows per core (256)
L = T - 1
NBLK = NEG + 1         # 20 negative g-blocks + 1 positive block
BR = F * R             # rows per block (1024)
NA = NLOG + 1          # action vocab + padding row

_PROGRAM_CACHE = {}


# ----------------------------------------------------------------- host prep

def _prep_core(c, inputs, u_list, k_eff):
    acts = np.asarray(inputs["actions"])[..., 0]
    nd = np.asarray(inputs["not_dones"])[..., 0]
    ri = np.asarray(inputs["rnn_inputs"], np.float32)
    ro = np.asarray(inputs["rnn_outputs"], np.float32)
    ti = np.asarray(inputs["time_subsample"]).astype(np.int64)
    neg_idx = np.asarray(inputs["neg_idx"]).astype(np.int64)

    ns = slice(c * NPC, (c + 1) * NPC)
    idx = np.arange(k_eff)[:, None] + ti[None, :]          # (k_eff, S)

    act_ext = np.full((NPC, L + K), NLOG, np.int64)
    act_ext[:, :L] = acts[ns, :L]
    AI = act_ext[:, idx].transpose(1, 0, 2).reshape(k_eff, R)  # (k_eff, R)

    # one-hot actions: oh[a, k, r] = (AI[k, r] == a)
    oh = np.zeros((NA, k_eff, R), np.float32)
    kk = np.repeat(np.arange(k_eff), R)
    rr = np.tile(np.arange(R), k_eff)
    oh[AI.reshape(-1), kk, rr] = 1.0
    ohT = oh.astype(BF16)

    # g-gate input-side term, gathered on host: x@W_ih_g.T + b_ih_g
    W_ih = np.asarray(inputs["W_ih"], np.float32)
    b_ih = np.asarray(inputs["b_ih"], np.float32)
    emb_tab = np.asarray(inputs["action_embed"], np.float32)
    GIE_G = np.zeros((NA, 512), np.float32)
    GIE_G[:NLOG] = emb_tab @ W_ih[1024:].T + b_ih[1024:]
    GIE_G[NLOG] = b_ih[1024:]
    gig = GIE_G[AI]                                        # (k_eff, R, 512)
    gigT = np.ascontiguousarray(
        gig.transpose(0, 2, 1).reshape(k_eff, 4, 128, R)).astype(BF16)

    H0 = ro[ns][:, ti]                                     # (NPC, S, H)
    h0T = np.ascontiguousarray(
        H0.transpose(2, 0, 1).reshape(4, 128, R)).astype(BF16)

    ri_ext = np.zeros((NPC, L + K, H), np.float32)
    ri_ext[:, :L] = ri[ns, 1:]
    idx2 = np.asarray(u_list)[:, None] + ti[None, :]       # (F, S)
    TG = ri_ext[:, idx2]                                   # (NPC, F, S, H)
    tgT = np.ascontiguousarray(
        TG.transpose(3, 1, 0, 2).reshape(H, BR).reshape(4, 128, BR)).astype(F8)

    ni = neg_idx.reshape(F, N, S, NEG)[:, ns]              # (F, NPC, S, NEG)
    P = ni.transpose(3, 0, 1, 2).reshape(-1)               # cols in (g, f, j) order
    negs = ri.reshape(N * T, H)[P]
    negsT = np.ascontiguousarray(negs.T.reshape(4, 128, NEG * BR)).astype(F8)

    nd_ext = np.zeros((NPC, L + K), np.float32)
    nd_ext[:, :L] = nd[ns, :L]
    G = nd_ext[:, idx]                                     # (NPC, k_eff, S)
    ndv = G.transpose(1, 0, 2).reshape(k_eff, R)
    ndvT = np.ascontiguousarray(
        ndv.reshape(k_eff, 2, 128).transpose(2, 0, 1)).astype(np.float32)

    return dict(ohT=ohT, gigT=gigT, h0T=h0T, tgT=tgT, negsT=negsT, ndvT=ndvT)


def _prep_weights(inputs):
    W_ih = np.asarray(inputs["W_ih"], np.float32)
    W_hh = np.asarray(inputs["W_hh"], np.float32)
    b_ih = np.asarray(inputs["b_ih"], np.float32)
    b_hh = np.asarray(inputs["b_hh"], np.float32)
    emb_tab = np.asarray(inputs["action_embed"], np.float32)
    W1 = np.asarray(inputs["W1"], np.float32)
    b1 = np.asarray(inputs["b1"], np.float32)
    W2 = np.asarray(inputs["W2"], np.float32)
    b2 = np.asarray(inputs["b2"], np.float32)
    W3 = np.asarray(inputs["W3"], np.float32)
    b3 = np.asarray(inputs["b3"], np.float32)

    d = {}
    # W_hh.T laid out [kc, 128, 1536] bf16 (stationary chunks per H-chunk kc)
    d["whhT"] = np.ascontiguousarray(
        W_hh.T.reshape(4, 128, 1536)).astype(BF16)
    # gie: 19-row table. cols 0:1024 = emb@W_ih_rz.T + b_ih_rz + b_hh_rz
    # (complete r/z input-side term); cols 1024:1536 = b_hh_g broadcast
    # (all rows identical -> one-hot matmul adds b_hh_g to the g psum).
    gie = np.zeros((NA, 1536), np.float32)
    gie[:NLOG, :1024] = emb_tab @ W_ih[:1024].T + b_ih[:1024] + b_hh[:1024]
    gie[NLOG, :1024] = b_ih[:1024] + b_hh[:1024]
    gie[:, 1024:] = b_hh[1024:][None, :]
    d["gie"] = gie.astype(BF16)

    def pack8(WT):
        # [t, ki, ko, m] with contract index = t*256 + ko*128 + ki
        return np.ascontiguousarray(
            WT.reshape(2, 2, 128, WT.shape[1]).transpose(0, 2, 1, 3)).astype(F8)
    d["w1a8"] = pack8(W1[:, :512].T.copy())
    d["w1b8"] = pack8(W1[:, 512:].T.copy())
    d["w28"] = pack8(W2.T.copy())
    d["w3T"] = np.ascontiguousarray(W3[0].reshape(4, 128).T).astype(BF16)
    d["b1T"] = np.ascontiguousarray(b1.reshape(4, 128).T).astype(np.float32)
    d["b2T"] = np.ascontiguousarray(b2.reshape(4, 128).T).astype(np.float32)
    d["b3f"] = float(b3.reshape(-1)[0])
    return d


# ------------------------------------------------------------- device program

def _gsl(t, c):
    """[128, 2, 512] tile viewed as 4 chunks of [128, 256]."""
    return t[:, c // 2, (c % 2) * 256:(c % 2) * 256 + 256]


def _build_program(u_list, k_eff, b3f):
    nc = bacc.Bacc("TRN2", target_bir_lowering=False, debug=False, num_devices=NC)

    di = {}
    def inp(name, shape, dt):
        di[name] = nc.dram_tensor(name, list(shape), dt, kind="ExternalInput")
        return di[name]

    d_whh = inp("whhT", (4, 128, 1536), DT.bfloat16)
    d_gie = inp("gie", (NA, 1536), DT.bfloat16)
    d_oh = inp("ohT", (NA, k_eff, R), DT.bfloat16)
    d_gig = inp("gigT", (k_eff, 4, 128, R), DT.bfloat16)
    d_w1a = inp("w1a8", (2, 128, 2, 512), DT.float8e4)
    d_w1b = inp("w1b8", (2, 128, 2, 512), DT.float8e4)
    d_w2 = inp("w28", (2, 128, 2, 512), DT.float8e4)
    d_w3 = inp("w3T", (128, 4), DT.bfloat16)
    d_b1 = inp("b1T", (128, 4), DT.float32)
    d_b2 = inp("b2T", (128, 4), DT.float32)
    d_h0 = inp("h0T", (4, 128, R), DT.bfloat16)
    d_tg = inp("tgT", (4, 128, BR), DT.float8e4)
    d_negs = inp("negsT", (4, 128, NEG * BR), DT.float8e4)
    d_ndv = inp("ndvT", (128, k_eff, 2), DT.float32)
    d_out = nc.dram_tensor("out", [1, 4], DT.float32, kind="ExternalOutput")

    f32 = DT.float32
    bf16 = DT.bfloat16
    f8 = DT.float8e4

    with tile.TileContext(nc) as tc:
        with (
            tc.tile_pool(name="const", bufs=1) as cp,
            tc.tile_pool(name="gruw", bufs=2) as gp,
            tc.tile_pool(name="mlpw", bufs=3) as mp,
            tc.tile_pool(name="ps", bufs=4, space="PSUM") as pp,
        ):
            # ------------------------------------------------ constant loads
            whh = cp.tile([128, 4, 1536], bf16, tag="whh")
            for kc in range(4):
                nc.sync.dma_start(out=whh[:, kc, :], in_=d_whh[kc])
            gie = cp.tile([NA, 1536], bf16, tag="gie")
            nc.sync.dma_start(out=gie[:], in_=d_gie[:])
            oh = cp.tile([NA, k_eff, R], bf16, tag="oh")
            nc.sync.dma_start(out=oh[:], in_=d_oh[:])
            w1a = cp.tile([128, 2, 2, 512], f8, tag="w1a")
            w1b = cp.tile([128, 2, 2, 512], f8, tag="w1b")
            w2 = cp.tile([128, 2, 2, 512], f8, tag="w2")
            for (t, d) in ((w1a, d_w1a), (w1b, d_w1b), (w2, d_w2)):
                for th in range(2):
                    nc.sync.dma_start(out=t[:, th, :, :], in_=d[th])
            w3 = cp.tile([128, 4], bf16, tag="w3")
            nc.sync.dma_start(out=w3[:], in_=d_w3[:])
            b1 = cp.tile([128, 4], f32, tag="b1")
            nc.sync.dma_start(out=b1[:], in_=d_b1[:])
            b2 = cp.tile([128, 4], f32, tag="b2")
            nc.sync.dma_start(out=b2[:], in_=d_b2[:])
            tg = cp.tile([128, 4, BR], f8, tag="tg")
            for kc in range(4):
                nc.sync.dma_start(out=tg[:, kc, :], in_=d_tg[kc])
            ndv = cp.tile([128, k_eff, 2], f32, tag="ndv")
            nc.sync.dma_start(out=ndv[:], in_=d_ndv[:])

            # ------------------------------------------------ forward mask
            prod = cp.tile([128, k_eff, 2], f32, tag="prod")
            nc.vector.tensor_scalar(prod[:, 0, :], ndv[:, 0, :], 0.0, None,
                                    op0=ALU.is_gt)
            for k in range(1, k_eff):
                nc.vector.scalar_tensor_tensor(
                    prod[:, k, :], in0=ndv[:, k, :], scalar=0.0,
                    in1=prod[:, k - 1, :], op0=ALU.is_gt, op1=ALU.mult)
            mfT = cp.tile([128, 2 * F], f32, tag="mfT")
            for fi, u in enumerate(u_list):
                nc.vector.tensor_copy(mfT[:, 2 * fi:2 * fi + 2], prod[:, u, :])

            # ------------------------------------------------ GRU (bf16)
            # Gate order in the 1536 dim: r(512), z(512), g(512).
            # Per step: 48 W_hh matmuls + 12 one-hot matmuls (gi / biases
            # folded into PSUM).  r/z sigmoid straight from PSUM on the
            # scalar engine; g tail per 128-H-chunk so the next step's
            # matmuls start as soon as chunk 0 of h_new is written.
            h_prev = gp.tile([128, 4, R], bf16, tag="h")
            for kc in range(4):
                nc.sync.dma_start(out=h_prev[:, kc, :], in_=d_h0[kc])
            predsT = cp.tile([128, 4, BR], f8, tag="preds")

            for k in range(k_eff):
                gig = gp.tile([128, 4, R], bf16, tag="gig", bufs=3)
                for kc in range(4):
                    nc.sync.dma_start(out=gig[:, kc, :], in_=d_gig[k, kc])

                ps_r = pp.tile([128, 2, 512], f32, tag="ps")
                ps_g = pp.tile([128, 2, 512], f32, tag="ps")
                ps_z = pp.tile([128, 2, 512], f32, tag="ps")
                r_sb = gp.tile([128, 4, R], bf16, tag="r")
                z_sb = gp.tile([128, 4, R], bf16, tag="z")
                t_sb = gp.tile([128, 4, R], bf16, tag="t")
                u_sb = gp.tile([128, 4, R], bf16, tag="u")
                g_sb = gp.tile([128, 4, R], bf16, tag="g")
                d_sb = gp.tile([128, 4, R], bf16, tag="d")
                hz_sb = gp.tile([128, 4, R], bf16, tag="hz")
                h_new = gp.tile([128, 4, R], bf16, tag="h")

                def gates(ps, goff):
                    # one gate group (4 chunks of 128 outputs) into psum
                    for c in range(4):
                        sl = slice((goff + c) * 128, (goff + c + 1) * 128)
                        for kc in range(4):
                            nc.tensor.matmul(
                                _gsl(ps, c), whh[:, kc, sl], h_prev[:, kc, :],
                                start=(kc == 0), stop=False)
                        nc.tensor.matmul(
                            _gsl(ps, c), gie[0:NA, sl], oh[0:NA, k, :],
                            start=False, stop=True)

                gates(ps_r, 0)
                for c in range(4):
                    nc.scalar.activation(r_sb[:, c, :], _gsl(ps_r, c),
                                         AF.Sigmoid)
                gates(ps_g, 8)
                gates(ps_z, 4)
                for c in range(4):
                    # t = (W_hh_g@h + b_hh_g) * r ; u = gi_g + t ; g = tanh(u)
                    nc.vector.tensor_mul(t_sb[:, c, :], _gsl(ps_g, c),
                                         r_sb[:, c, :])
                    nc.vector.tensor_add(u_sb[:, c, :], gig[:, c, :],
                                         t_sb[:, c, :])
                    nc.scalar.activation(g_sb[:, c, :], u_sb[:, c, :], AF.Tanh)
                    nc.scalar.activation(z_sb[:, c, :], _gsl(ps_z, c),
                                         AF.Sigmoid)
                    # h_new = g + z*(h - g)
                    nc.vector.tensor_sub(d_sb[:, c, :], h_prev[:, c, :],
                                         g_sb[:, c, :])
                    nc.vector.tensor_mul(hz_sb[:, c, :], z_sb[:, c, :],
                                         d_sb[:, c, :])
                    nc.vector.tensor_add(h_new[:, c, :], g_sb[:, c, :],
                                         hz_sb[:, c, :])
                h_prev = h_new
                for fi, u in enumerate(u_list):
                    if u == k:
                        nc.vector.tensor_copy(
                            predsT[:, :, fi * R:(fi + 1) * R], h_new[:])

            # ------------------------------------------------ p1 = preds@W1a + b1
            DR = mybir.MatmulPerfMode.DoubleRow
            p1e = cp.tile([128, 4, BR], bf16, tag="p1e")
            for cc in range(4):
                ps = pp.tile([128, 2, 512], f32, tag="ps")
                for rt in range(2):
                    sl = slice(rt * 512, (rt + 1) * 512)
                    for th in range(2):
                        nc.tensor.matmul(
                            ps[:, rt, :],
                            w1a[:, th, :, cc * 128:(cc + 1) * 128],
                            predsT[:, 2 * th:2 * th + 2, sl],
                            start=(th == 0), stop=(th == 1), perf_mode=DR)
                nc.scalar.activation(p1e[:, cc, :], ps[:], AF.Identity,
                                     bias=b1[:, cc:cc + 1])

            # ------------------------------------------------ blocks
            # L1: only x@W1b on the tensor engine; eviction adds p1e
            # (vector STT) and applies ReLU (scalar for cc 0/1, vector for
            # cc 2/3 to balance the two engines).
            logits = cp.tile([128, NBLK, 8], f32, tag="logits")
            for b in range(NBLK):
                if b < NEG:
                    xt = mp.tile([128, 4, BR], f8, tag="negsx")
                    for kc in range(4):
                        nc.sync.dma_start(
                            out=xt[:, kc, :],
                            in_=d_negs[kc][:, b * BR:(b + 1) * BR])
                else:
                    xt = tg
                y1 = mp.tile([128, 4, BR], f8, tag="y1", bufs=2)
                y1p = mp.tile([128, 4, BR], bf16, tag="y1p", bufs=2)
                for cc in range(4):
                    ps = pp.tile([128, 2, 512], f32, tag="ps")
                    for rt in range(2):
                        sl = slice(rt * 512, (rt + 1) * 512)
                        for th in range(2):
                            nc.tensor.matmul(
                                ps[:, rt, :],
                                w1b[:, th, :, cc * 128:(cc + 1) * 128],
                                xt[:, 2 * th:2 * th + 2, sl],
                                start=(th == 0), stop=(th == 1), perf_mode=DR)
                    nc.vector.scalar_tensor_tensor(
                        y1p[:, cc, :], in0=ps[:], scalar=0.0,
                        in1=p1e[:, cc, :], op0=ALU.add, op1=ALU.add)
                    if cc < 2:
                        nc.scalar.activation(y1[:, cc, :], y1p[:, cc, :],
                                             AF.Relu)
                    else:
                        nc.vector.tensor_scalar(y1[:, cc, :], y1p[:, cc, :],
                                                0.0, None, op0=ALU.max)
                y2 = mp.tile([128, 4, BR], bf16, tag="y2", bufs=2)
                for cc in range(4):
                    ps = pp.tile([128, 2, 512], f32, tag="ps")
                    for rt in range(2):
                        sl = slice(rt * 512, (rt + 1) * 512)
                        for th in range(2):
                            nc.tensor.matmul(
                                ps[:, rt, :],
                                w2[:, th, :, cc * 128:(cc + 1) * 128],
                                y1[:, 2 * th:2 * th + 2, sl],
                                start=(th == 0), stop=(th == 1), perf_mode=DR)
                    nc.scalar.activation(y2[:, cc, :], ps[:], AF.Relu,
                                         bias=b2[:, cc:cc + 1])
                ps3 = pp.tile([128, 2, 512], f32, tag="ps")
                for col in range(8):
                    for kc in range(4):
                        nc.tensor.matmul(
                            ps3[:, 0, col:col + 1],
                            y2[:, kc, col * 128:(col + 1) * 128],
                            w3[:, kc:kc + 1], start=(kc == 0), stop=(kc == 3))
                nc.scalar.activation(logits[:, b, :], ps3[:, 0, 0:8], AF.Copy)

            # ------------------------------------- softplus + sums
            # softplus(t) = relu(t) - ln(sigmoid(|t|)); whole-tensor ACT ops
            # keep the activation-table sequence to a single switch.
            partials = cp.tile([128, NBLK + 1], f32, tag="partials")
            sp_a = cp.tile([128, NBLK, 8], f32, tag="sp_a")
            sp_l = cp.tile([128, NBLK, 8], f32, tag="sp_l")
            sp_r = cp.tile([128, NBLK, 8], f32, tag="sp_r")
            sp_d = cp.tile([128, 8], f32, tag="sp_d")
            nc.scalar.activation(sp_a[:], logits[:], AF.Abs, bias=b3f)
            nc.scalar.activation(sp_a[:], sp_a[:], AF.Sigmoid)
            nc.scalar.activation(sp_l[:], sp_a[:], AF.Ln)
            nc.scalar.activation(sp_r[:, :NEG, :], logits[:, :NEG, :],
                                 AF.Relu, bias=b3f)
            nc.scalar.activation(sp_r[:, NEG, :], logits[:, NEG, :],
                                 AF.Relu, bias=-b3f, scale=-1.0)
            nc.vector.tensor_sub(sp_r[:], sp_r[:], sp_l[:])
            for b in range(NBLK):
                nc.vector.tensor_mul(sp_d[:], sp_r[:, b, :], mfT[:])
                nc.vector.tensor_reduce(partials[:, b:b + 1], sp_d[:],
                                        mybir.AxisListType.X, ALU.add)
            nc.vector.tensor_reduce(partials[:, NBLK:NBLK + 1], mfT[:],
                                    mybir.AxisListType.X, ALU.add)

            vcol = cp.tile([128, 4], f32, tag="vcol")
            nc.vector.tensor_copy(vcol[:, 0:1], partials[:, NEG:NEG + 1])
            nc.vector.tensor_reduce(vcol[:, 1:2], partials[:, 0:NEG],
                                    mybir.AxisListType.X, ALU.add)
            nc.vector.tensor_copy(vcol[:, 2:3], partials[:, NBLK:NBLK + 1])
            nc.any.memset(vcol[:, 3:4], 0.0)
            ones = cp.tile([128, 1], f32, tag="ones")
            nc.any.memset(ones[:], 1.0)
            psf = pp.tile([128, 2, 512], f32, tag="ps")
            nc.tensor.matmul(psf[0:1, 0, 0:4], ones[:], vcol[:],
                             start=True, stop=True)
            out_sb = cp.tile([1, 4], f32, tag="out_sb")
            nc.scalar.activation(out_sb[:], psf[0:1, 0, 0:4], AF.Copy)
            nc.sync.dma_start(out=d_out[:], in_=out_sb[:])

    nc.finalize()
    return nc


def _get_program(u_list, k_eff, b3f):
    key = (tuple(u_list), k_eff, float(b3f))
    if key not in _PROGRAM_CACHE:
        _PROGRAM_CACHE[key] = _build_program(u_list, k_eff, b3f)
    return _PROGRAM_CACHE[key]


# ------------------------------------------------------------------ kernel

def kernel(**inputs):
    u_list = [int(x) for x in np.asarray(inputs["unroll_subsample"]).reshape(-1)]
    k_eff = max(u_list) + 1
    w = _prep_weights(inputs)
    nc = _get_program(u_list, k_eff, w["b3f"])

    wmaps = {k: v for k, v in w.items() if k != "b3f"}
    in_maps = []
    for c in range(NC):
        m = dict(wmaps)
        m.update(_prep_core(c, inputs, u_list, k_eff))
        in_maps.append(m)

    res = bass_utils.run_bass_kernel_spmd(nc, in_maps, list(range(NC)))
    P = Ng = D = 0.0
    for c in range(NC):
        o = np.asarray(res.results[c]["out"], np.float64)
        P += o[0, 0]
        Ng += o[0, 1]
        D += o[0, 2]
    loss = COEFF * (P / D + Ng / (D * NEG))
    return np.float32(loss)


# revision 4
# speedup vs baseline: 1.1952x; 1.1952x over previous
"""Trainium2 Bass kernel for the CPCA auxiliary loss (nn_CPCA_51754355917033).

Strategy (data-parallel over the env/batch dim n, 16 envs per core):
  - GRU runs fully in bf16.  The input-side gate terms (x@W_ih.T + biases)
    are folded into the same PSUM accumulation as W_hh@h via one-hot action
    matmuls against a 19-row table (emb@W_ih.T + b_ih [+ b_hh]), so the only
    per-step vector work is the g-gate tail; r/z evict straight from PSUM
    through the scalar engine's sigmoid.  The gate tail is split into four
    128-row H-chunks so the next step's matmuls (which contract one H-chunk
    each) start as soon as the first chunk of h is ready.
  - MLP: preds@W1a + b1 is hoisted out of the 21-block loop (computed once,
    stashed as bf16 "p1e"); per block only x@W1b runs on the tensor engine
    and the eviction adds p1e (vector) + ReLU (scalar/vector split).
  - Host combines the 8 cores' (pos_sum, neg_sum, denom) partials into the
    final scalar (the all-reduce of the sharding hint, done at unshard time).
"""

import numpy as np
import ml_dtypes

import concourse.bass as bass
import concourse.mybir as mybir
import concourse.tile as tile
from concourse import bacc
from concourse import bass_utils

BF16 = ml_dtypes.bfloat16
F8 = ml_dtypes.float8_e4m3
DT = mybir.dt
AF = mybir.ActivationFunctionType
ALU = mybir.AluOpType

N, T, H, K, S, F, EMB, NLOG, NEG = 128, 512, 512, 16, 16, 4, 32, 18, 20
COEFF = 0.1
NC = 8
NPC = N // NC          # envs per core
R = NPC * S            # GRU rows per core (256)
L = T - 1
NBLK = NEG + 1         # 20 negative g-blocks + 1 positive block
BR = F * R             # rows per block (1024)
NA = NLOG + 1          # action vocab + padding row

_PROGRAM_CACHE = {}


# ----------------------------------------------------------------- host prep

def _prep_core(c, inputs, u_list, k_eff):
    acts = np.asarray(inputs["actions"])[..., 0]
    nd = np.asarray(inputs["not_dones"])[..., 0]
    ri = np.asarray(inputs["rnn_inputs"], np.float32)
    ro = np.asarray(inputs["rnn_outputs"], np.float32)
    ti = np.asarray(inputs["time_subsample"]).astype(np.int64)
    neg_idx = np.asarray(inputs["neg_idx"]).astype(np.int64)

    ns = slice(c * NPC, (c + 1) * NPC)
    idx = np.arange(k_eff)[:, None] + ti[None, :]          # (k_eff, S)

    act_ext = np.full((NPC, L + K), NLOG, np.int64)
    act_ext[:, :L] = acts[ns, :L]
    AI = act_ext[:, idx].transpose(1, 0, 2).reshape(k_eff, R)  # (k_eff, R)

    # one-hot actions padded to 128 partitions: oh[a, k, r] = (AI[k, r] == a)
    oh = np.zeros((128, k_eff, R), np.float32)
    kk = np.repeat(np.arange(k_eff), R)
    rr = np.tile(np.arange(R), k_eff)
    oh[AI.reshape(-1), kk, rr] = 1.0
    ohT = oh.astype(BF16)

    # g-gate input-side term, gathered on host: x@W_ih_g.T + b_ih_g
    W_ih = np.asarray(inputs["W_ih"], np.float32)
    b_ih = np.asarray(inputs["b_ih"], np.float32)
    emb_tab = np.asarray(inputs["action_embed"], np.float32)
    GIE_G = np.zeros((NA, 512), np.float32)
    GIE_G[:NLOG] = emb_tab @ W_ih[1024:].T + b_ih[1024:]
    GIE_G[NLOG] = b_ih[1024:]
    gig = GIE_G[AI]                                        # (k_eff, R, 512)
    gigT = np.ascontiguousarray(
        gig.transpose(0, 2, 1).reshape(k_eff, 4, 128, R)).astype(BF16)

    H0 = ro[ns][:, ti]                                     # (NPC, S, H)
    h0T = np.ascontiguousarray(
        H0.transpose(2, 0, 1).reshape(4, 128, R)).astype(BF16)

    ri_ext = np.zeros((NPC, L + K, H), np.float32)
    ri_ext[:, :L] = ri[ns, 1:]
    idx2 = np.asarray(u_list)[:, None] + ti[None, :]       # (F, S)
    TG = ri_ext[:, idx2]                                   # (NPC, F, S, H)
    tgT = np.ascontiguousarray(
        TG.transpose(3, 1, 0, 2).reshape(H, BR).reshape(4, 128, BR)).astype(F8)

    ni = neg_idx.reshape(F, N, S, NEG)[:, ns]              # (F, NPC, S, NEG)
    P = ni.transpose(3, 0, 1, 2).reshape(-1)               # cols in (g, f, j) order
    negs = ri.reshape(N * T, H)[P]
    negsT = np.ascontiguousarray(negs.T.reshape(4, 128, NEG * BR)).astype(F8)

    nd_ext = np.zeros((NPC, L + K), np.float32)
    nd_ext[:, :L] = nd[ns, :L]
    G = nd_ext[:, idx]                                     # (NPC, k_eff, S)
    ndv = G.transpose(1, 0, 2).reshape(k_eff, R)
    ndvT = np.ascontiguousarray(
        ndv.reshape(k_eff, 2, 128).transpose(2, 0, 1)).astype(np.float32)

    return dict(ohT=ohT, gigT=gigT, h0T=h0T, tgT=tgT, negsT=negsT, ndvT=ndvT)


def _prep_weights(inputs):
    W_ih = np.asarray(inputs["W_ih"], np.float32)
    W_hh = np.asarray(inputs["W_hh"], np.float32)
    b_ih = np.asarray(inputs["b_ih"], np.float32)
    b_hh = np.asarray(inputs["b_hh"], np.float32)
    emb_tab = np.asarray(inputs["action_embed"], np.float32)
    W1 = np.asarray(inputs["W1"], np.float32)
    b1 = np.asarray(inputs["b1"], np.float32)
    W2 = np.asarray(inputs["W2"], np.float32)
    b2 = np.asarray(inputs["b2"], np.float32)
    W3 = np.asarray(inputs["W3"], np.float32)
    b3 = np.asarray(inputs["b3"], np.float32)

    d = {}
    # W_hh.T packed for fp8 DoubleRow: [th, ki, ko, 1536]
    d["whh8"] = np.ascontiguousarray(
        W_hh.T.reshape(2, 2, 128, 1536).transpose(0, 2, 1, 3)).astype(F8)
    # gie: action table, padded to 128 rows and chunked contiguously
    # ([128, 12, 128] so each stationary slice is contiguous -> FWL).
    # cols 0:1024 = emb@W_ih_rz.T + b_ih_rz + b_hh_rz (complete r/z
    # input-side term); cols 1024:1536 = b_hh_g broadcast (all rows
    # identical -> the one-hot matmul adds b_hh_g to the g psum).
    gie = np.zeros((128, 1536), np.float32)
    gie[:NLOG, :1024] = emb_tab @ W_ih[:1024].T + b_ih[:1024] + b_hh[:1024]
    gie[NLOG, :1024] = b_ih[:1024] + b_hh[:1024]
    gie[:NA, 1024:] = b_hh[1024:][None, :]
    d["gie"] = np.ascontiguousarray(
        gie.reshape(128, 12, 128)).astype(BF16)

    def pack8(WT):
        # [t, ki, ko, m] with contract index = t*256 + ko*128 + ki
        return np.ascontiguousarray(
            WT.reshape(2, 2, 128, WT.shape[1]).transpose(0, 2, 1, 3)).astype(F8)
    d["w1a8"] = pack8(W1[:, :512].T.copy())
    d["w1b8"] = pack8(W1[:, 512:].T.copy())
    d["w28"] = pack8(W2.T.copy())
    d["w3T"] = np.ascontiguousarray(W3[0].reshape(4, 128).T).astype(BF16)
    d["b1T"] = np.ascontiguousarray(b1.reshape(4, 128).T).astype(np.float32)
    d["b2T"] = np.ascontiguousarray(b2.reshape(4, 128).T).astype(np.float32)
    d["b3f"] = float(b3.reshape(-1)[0])
    return d


# ------------------------------------------------------------- device program

def _gsl(t, c):
    """[128, 2, 512] tile viewed as 4 chunks of [128, 256]."""
    return t[:, c // 2, (c % 2) * 256:(c % 2) * 256 + 256]


def _build_program(u_list, k_eff, b3f):
    nc = bacc.Bacc("TRN2", target_bir_lowering=False, debug=False, num_devices=NC)

    di = {}
    def inp(name, shape, dt):
        di[name] = nc.dram_tensor(name, list(shape), dt, kind="ExternalInput")
        return di[name]

    d_whh = inp("whh8", (2, 128, 2, 1536), DT.float8e4)
    d_gie = inp("gie", (128, 12, 128), DT.bfloat16)
    d_oh = inp("ohT", (128, k_eff, R), DT.bfloat16)
    d_gig = inp("gigT", (k_eff, 4, 128, R), DT.bfloat16)
    d_w1a = inp("w1a8", (2, 128, 2, 512), DT.float8e4)
    d_w1b = inp("w1b8", (2, 128, 2, 512), DT.float8e4)
    d_w2 = inp("w28", (2, 128, 2, 512), DT.float8e4)
    d_w3 = inp("w3T", (128, 4), DT.bfloat16)
    d_b1 = inp("b1T", (128, 4), DT.float32)
    d_b2 = inp("b2T", (128, 4), DT.float32)
    d_h0 = inp("h0T", (4, 128, R), DT.bfloat16)
    d_tg = inp("tgT", (4, 128, BR), DT.float8e4)
    d_negs = inp("negsT", (4, 128, NEG * BR), DT.float8e4)
    d_ndv = inp("ndvT", (128, k_eff, 2), DT.float32)
    d_out = nc.dram_tensor("out", [1, 4], DT.float32, kind="ExternalOutput")

    f32 = DT.float32
    bf16 = DT.bfloat16
    f8 = DT.float8e4

    with tile.TileContext(nc) as tc:
        with (
            tc.tile_pool(name="const", bufs=1) as cp,
            tc.tile_pool(name="gruw", bufs=2) as gp,
            tc.tile_pool(name="mlpw", bufs=3) as mp,
            tc.tile_pool(name="ps", bufs=4, space="PSUM") as pp,
        ):
            # ------------------------------------------------ constant loads
            whh = cp.tile([128, 2, 2, 1536], f8, tag="whh")
            for th in range(2):
                nc.sync.dma_start(out=whh[:, th, :, :], in_=d_whh[th])
            gie = cp.tile([128, 12, 128], bf16, tag="gie")
            nc.sync.dma_start(out=gie[:], in_=d_gie[:])
            oh = cp.tile([128, k_eff, R], bf16, tag="oh")
            nc.sync.dma_start(out=oh[:], in_=d_oh[:])
            w1a = cp.tile([128, 2, 2, 512], f8, tag="w1a")
            w1b = cp.tile([128, 2, 2, 512], f8, tag="w1b")
            w2 = cp.tile([128, 2, 2, 512], f8, tag="w2")
            for (t, d) in ((w1a, d_w1a), (w1b, d_w1b), (w2, d_w2)):
                for th in range(2):
                    nc.sync.dma_start(out=t[:, th, :, :], in_=d[th])
            w3 = cp.tile([128, 4], bf16, tag="w3")
            nc.sync.dma_start(out=w3[:], in_=d_w3[:])
            b1 = cp.tile([128, 4], f32, tag="b1")
            nc.sync.dma_start(out=b1[:], in_=d_b1[:])
            b2 = cp.tile([128, 4], f32, tag="b2")
            nc.sync.dma_start(out=b2[:], in_=d_b2[:])
            tg = cp.tile([128, 4, BR], f8, tag="tg")
            for kc in range(4):
                nc.sync.dma_start(out=tg[:, kc, :], in_=d_tg[kc])
            ndv = cp.tile([128, k_eff, 2], f32, tag="ndv")
            nc.sync.dma_start(out=ndv[:], in_=d_ndv[:])

            # ------------------------------------------------ forward mask
            prod = cp.tile([128, k_eff, 2], f32, tag="prod")
            nc.vector.tensor_scalar(prod[:, 0, :], ndv[:, 0, :], 0.0, None,
                                    op0=ALU.is_gt)
            for k in range(1, k_eff):
                nc.vector.scalar_tensor_tensor(
                    prod[:, k, :], in0=ndv[:, k, :], scalar=0.0,
                    in1=prod[:, k - 1, :], op0=ALU.is_gt, op1=ALU.mult)
            mfT = cp.tile([128, 2 * F], f32, tag="mfT")
            for fi, u in enumerate(u_list):
                nc.vector.tensor_copy(mfT[:, 2 * fi:2 * fi + 2], prod[:, u, :])

            # ------------------------------------------------ GRU (bf16)
            # Gate order in the 1536 dim: r(512), z(512), g(512).
            # Per step: 48 W_hh matmuls + 12 one-hot matmuls (gi / biases
            # folded into PSUM).  r/z sigmoid straight from PSUM on the
            # scalar engine; g tail per 128-H-chunk so the next step's
            # matmuls start as soon as chunk 0 of h_new is written.
            h_prev = gp.tile([128, 4, R], bf16, tag="h")
            for kc in range(4):
                nc.sync.dma_start(out=h_prev[:, kc, :], in_=d_h0[kc])
            h8_prev = gp.tile([128, 4, R], f8, tag="h8")
            nc.vector.tensor_copy(h8_prev[:], h_prev[:])
            predsT = cp.tile([128, 4, BR], f8, tag="preds")
            DRM = mybir.MatmulPerfMode.DoubleRow

            for k in range(k_eff):
                gig = gp.tile([128, 4, R], bf16, tag="gig", bufs=3)
                for kc in range(4):
                    nc.sync.dma_start(out=gig[:, kc, :], in_=d_gig[k, kc])

                ps_r = pp.tile([128, 2, 512], f32, tag="ps")
                ps_g = pp.tile([128, 2, 512], f32, tag="ps")
                ps_z = pp.tile([128, 2, 512], f32, tag="ps")
                r_sb = gp.tile([128, 4, R], bf16, tag="r")
                z_sb = gp.tile([128, 4, R], bf16, tag="z")
                t_sb = gp.tile([128, 4, R], bf16, tag="t")
                u_sb = gp.tile([128, 4, R], bf16, tag="u")
                g_sb = gp.tile([128, 4, R], bf16, tag="g")
                d_sb = gp.tile([128, 4, R], bf16, tag="d")
                hz_sb = gp.tile([128, 4, R], bf16, tag="hz")
                h_new = gp.tile([128, 4, R], bf16, tag="h")
                h8_new = gp.tile([128, 4, R], f8, tag="h8")

                def gates(ps, goff):
                    # one gate group (4 chunks of 128 outputs) into psum:
                    # 2 fp8 DoubleRow W_hh matmuls + 1 bf16 one-hot matmul
                    # (input-side gi / bias term folded into the psum).
                    for c in range(4):
                        gc = goff + c
                        sl = slice(gc * 128, (gc + 1) * 128)
                        for th in range(2):
                            nc.tensor.matmul(
                                _gsl(ps, c), whh[:, th, :, sl],
                                h8_prev[:, 2 * th:2 * th + 2, :],
                                start=(th == 0), stop=False, perf_mode=DRM)
                        nc.tensor.matmul(
                            _gsl(ps, c), gie[:, gc, :], oh[:, k, :],
                            start=False, stop=True)

                gates(ps_r, 0)
                for c in range(4):
                    nc.scalar.activation(r_sb[:, c, :], _gsl(ps_r, c),
                                         AF.Sigmoid)
                gates(ps_g, 8)
                gates(ps_z, 4)
                for c in range(4):
                    # t = (W_hh_g@h + b_hh_g) * r ; u = gi_g + t ; g = tanh(u)
                    nc.vector.tensor_mul(t_sb[:, c, :], _gsl(ps_g, c),
                                         r_sb[:, c, :])
                    nc.vector.tensor_add(u_sb[:, c, :], gig[:, c, :],
                                         t_sb[:, c, :])
                    nc.scalar.activation(g_sb[:, c, :], u_sb[:, c, :], AF.Tanh)
                    nc.scalar.activation(z_sb[:, c, :], _gsl(ps_z, c),
                                         AF.Sigmoid)
                    # h_new = g + z*(h - g)
                    nc.vector.tensor_sub(d_sb[:, c, :], h_prev[:, c, :],
                                         g_sb[:, c, :])
                    nc.vector.tensor_mul(hz_sb[:, c, :], z_sb[:, c, :],
                                         d_sb[:, c, :])
                    nc.vector.tensor_add(h_new[:, c, :], g_sb[:, c, :],
                                         hz_sb[:, c, :])
                nc.vector.tensor_copy(h8_new[:, 0:2, :], h_new[:, 0:2, :])
                nc.scalar.activation(h8_new[:, 2:4, :], h_new[:, 2:4, :],
                                     AF.Copy)
                h_prev = h_new
                h8_prev = h8_new
                for fi, u in enumerate(u_list):
                    if u == k:
                        nc.vector.tensor_copy(
                            predsT[:, :, fi * R:(fi + 1) * R], h_new[:])

            # ------------------------------------------------ p1 = preds@W1a + b1
            DR = mybir.MatmulPerfMode.DoubleRow
            p1e = cp.tile([128, 4, BR], bf16, tag="p1e")
            for cc in range(4):
                ps = pp.tile([128, 2, 512], f32, tag="ps")
                for rt in range(2):
                    sl = slice(rt * 512, (rt + 1) * 512)
                    for th in range(2):
                        nc.tensor.matmul(
                            ps[:, rt, :],
                            w1a[:, th, :, cc * 128:(cc + 1) * 128],
                            predsT[:, 2 * th:2 * th + 2, sl],
                            start=(th == 0), stop=(th == 1), perf_mode=DR)
                nc.scalar.activation(p1e[:, cc, :], ps[:], AF.Identity,
                                     bias=b1[:, cc:cc + 1])

            # ------------------------------------------------ blocks
            # L1: only x@W1b on the tensor engine; eviction adds p1e
            # (vector STT) and applies ReLU (scalar for cc 0/1, vector for
            # cc 2/3 to balance the two engines).
            logits = cp.tile([128, NBLK, 8], f32, tag="logits")
            for b in range(NBLK):
                if b < NEG:
                    xt = mp.tile([128, 4, BR], f8, tag="negsx")
                    for kc in range(4):
                        nc.sync.dma_start(
                            out=xt[:, kc, :],
                            in_=d_negs[kc][:, b * BR:(b + 1) * BR])
                else:
                    xt = tg
                y1 = mp.tile([128, 4, BR], f8, tag="y1", bufs=2)
                y1p = mp.tile([128, 4, BR], bf16, tag="y1p", bufs=2)
                for cc in range(4):
                    ps = pp.tile([128, 2, 512], f32, tag="ps")
                    for rt in range(2):
                        sl = slice(rt * 512, (rt + 1) * 512)
                        for th in range(2):
                            nc.tensor.matmul(
                                ps[:, rt, :],
                                w1b[:, th, :, cc * 128:(cc + 1) * 128],
                                xt[:, 2 * th:2 * th + 2, sl],
                                start=(th == 0), stop=(th == 1), perf_mode=DR)
                    nc.vector.scalar_tensor_tensor(
                        y1p[:, cc, :], in0=ps[:], scalar=0.0,
                        in1=p1e[:, cc, :], op0=ALU.add, op1=ALU.add)
                    if cc < 2:
                        nc.scalar.activation(y1[:, cc, :], y1p[:, cc, :],
                                             AF.Relu)
                    else:
                        nc.vector.tensor_scalar(y1[:, cc, :], y1p[:, cc, :],
                                                0.0, None, op0=ALU.max)
                y2 = mp.tile([128, 4, BR], bf16, tag="y2", bufs=2)
                for cc in range(4):
                    ps = pp.tile([128, 2, 512], f32, tag="ps")
                    for rt in range(2):
                        sl = slice(rt * 512, (rt + 1) * 512)
                        for th in range(2):
                            nc.tensor.matmul(
                                ps[:, rt, :],
                                w2[:, th, :, cc * 128:(cc + 1) * 128],
                                y1[:, 2 * th:2 * th + 2, sl],
                                start=(th == 0), stop=(th == 1), perf_mode=DR)
                    nc.scalar.activation(y2[:, cc, :], ps[:], AF.Relu,
                                         bias=b2[:, cc:cc + 1])
                ps3 = pp.tile([128, 2, 512], f32, tag="ps")
                for col in range(8):
                    for kc in range(4):
                        nc.tensor.matmul(
                            ps3[:, 0, col:col + 1],
                            y2[:, kc, col * 128:(col + 1) * 128],
                            w3[:, kc:kc + 1], start=(kc == 0), stop=(kc == 3))
                nc.scalar.activation(logits[:, b, :], ps3[:, 0, 0:8], AF.Copy)

            # ------------------------------------- softplus + sums
            # softplus(t) = relu(t) - ln(sigmoid(|t|)); whole-tensor ACT ops
            # keep the activation-table sequence to a single switch.
            partials = cp.tile([128, NBLK + 1], f32, tag="partials")
            sp_a = cp.tile([128, NBLK, 8], f32, tag="sp_a")
            sp_l = cp.tile([128, NBLK, 8], f32, tag="sp_l")
            sp_r = cp.tile([128, NBLK, 8], f32, tag="sp_r")
            sp_d = cp.tile([128, 8], f32, tag="sp_d")
            nc.scalar.activation(sp_a[:], logits[:], AF.Abs, bias=b3f)
            nc.scalar.activation(sp_a[:], sp_a[:], AF.Sigmoid)
            nc.scalar.activation(sp_l[:], sp_a[:], AF.Ln)
            nc.scalar.activation(sp_r[:, :NEG, :], logits[:, :NEG, :],
                                 AF.Relu, bias=b3f)
            nc.scalar.activation(sp_r[:, NEG, :], logits[:, NEG, :],
                                 AF.Relu, bias=-b3f, scale=-1.0)
            nc.vector.tensor_sub(sp_r[:], sp_r[:], sp_l[:])
            for b in range(NBLK):
                nc.vector.tensor_mul(sp_d[:], sp_r[:, b, :], mfT[:])
                nc.vector.tensor_reduce(partials[:, b:b + 1], sp_d[:],
                                        mybir.AxisListType.X, ALU.add)
            nc.vector.tensor_reduce(partials[:, NBLK:NBLK + 1], mfT[:],
                                    mybir.AxisListType.X, ALU.add)

            vcol = cp.tile([128, 4], f32, tag="vcol")
            nc.vector.tensor_copy(vcol[:, 0:1], partials[:, NEG:NEG + 1])
            nc.vector.tensor_reduce(vcol[:, 1:2], partials[:, 0:NEG],
                                    mybir.AxisListType.X, ALU.add)
            nc.vector.tensor_copy(vcol[:, 2:3], partials[:, NBLK:NBLK + 1])
            nc.any.memset(vcol[:, 3:4], 0.0)
            ones = cp.tile([128, 1], f32, tag="ones")
            nc.any.memset(ones[:], 1.0)
            psf = pp.tile([128, 2, 512], f32, tag="ps")
            nc.tensor.matmul(psf[0:1, 0, 0:4], ones[:], vcol[:],
                             start=True, stop=True)
            out_sb = cp.tile([1, 4], f32, tag="out_sb")
            nc.scalar.activation(out_sb[:], psf[0:1, 0, 0:4], AF.Copy)
            nc.sync.dma_start(out=d_out[:], in_=out_sb[:])

    nc.finalize()
    return nc


def _get_program(u_list, k_eff, b3f):
    key = (tuple(u_list), k_eff, float(b3f))
    if key not in _PROGRAM_CACHE:
        _PROGRAM_CACHE[key] = _build_program(u_list, k_eff, b3f)
    return _PROGRAM_CACHE[key]


# ------------------------------------------------------------------ kernel

def kernel(**inputs):
    u_list = [int(x) for x in np.asarray(inputs["unroll_subsample"]).reshape(-1)]
    k_eff = max(u_list) + 1
    w = _prep_weights(inputs)
    nc = _get_program(u_list, k_eff, w["b3f"])

    wmaps = {k: v for k, v in w.items() if k != "b3f"}
    in_maps = []
    for c in range(NC):
        m = dict(wmaps)
        m.update(_prep_core(c, inputs, u_list, k_eff))
        in_maps.append(m)

    res = bass_utils.run_bass_kernel_spmd(nc, in_maps, list(range(NC)))
    P = Ng = D = 0.0
    for c in range(NC):
        o = np.asarray(res.results[c]["out"], np.float64)
        P += o[0, 0]
        Ng += o[0, 1]
        D += o[0, 2]
    loss = COEFF * (P / D + Ng / (D * NEG))
    return np.float32(loss)


# revision 5
# speedup vs baseline: 1.3441x; 1.1245x over previous
"""Trainium2 Bass kernel for the CPCA auxiliary loss (nn_CPCA_51754355917033).

Strategy (data-parallel over the env/batch dim n, 16 envs per core):
  - GRU runs fully in bf16.  The input-side gate terms (x@W_ih.T + biases)
    are folded into the same PSUM accumulation as W_hh@h via one-hot action
    matmuls against a 19-row table (emb@W_ih.T + b_ih [+ b_hh]), so the only
    per-step vector work is the g-gate tail; r/z evict straight from PSUM
    through the scalar engine's sigmoid.  The gate tail is split into four
    128-row H-chunks so the next step's matmuls (which contract one H-chunk
    each) start as soon as the first chunk of h is ready.
  - MLP: preds@W1a + b1 is hoisted out of the 21-block loop (computed once,
    stashed as bf16 "p1e"); per block only x@W1b runs on the tensor engine
    and the eviction adds p1e (vector) + ReLU (scalar/vector split).
  - Host combines the 8 cores' (pos_sum, neg_sum, denom) partials into the
    final scalar (the all-reduce of the sharding hint, done at unshard time).
"""

import numpy as np
import ml_dtypes

import concourse.bass as bass
import concourse.mybir as mybir
import concourse.tile as tile
from concourse import bacc
from concourse import bass_utils

BF16 = ml_dtypes.bfloat16
F8 = ml_dtypes.float8_e4m3
DT = mybir.dt
AF = mybir.ActivationFunctionType
ALU = mybir.AluOpType

N, T, H, K, S, F, EMB, NLOG, NEG = 128, 512, 512, 16, 16, 4, 32, 18, 20
COEFF = 0.1
NC = 8
NPC = N // NC          # envs per core
R = NPC * S            # GRU rows per core (256)
L = T - 1
NBLK = NEG + 1         # 20 negative g-blocks + 1 positive block
BR = F * R             # rows per block (1024)
NA = NLOG + 1          # action vocab + padding row

_PROGRAM_CACHE = {}


# ----------------------------------------------------------------- host prep

def _prep_core(c, inputs, u_list, k_eff):
    acts = np.asarray(inputs["actions"])[..., 0]
    nd = np.asarray(inputs["not_dones"])[..., 0]
    ri = np.asarray(inputs["rnn_inputs"], np.float32)
    ro = np.asarray(inputs["rnn_outputs"], np.float32)
    ti = np.asarray(inputs["time_subsample"]).astype(np.int64)
    neg_idx = np.asarray(inputs["neg_idx"]).astype(np.int64)

    ns = slice(c * NPC, (c + 1) * NPC)
    idx = np.arange(k_eff)[:, None] + ti[None, :]          # (k_eff, S)

    act_ext = np.full((NPC, L + K), NLOG, np.int64)
    act_ext[:, :L] = acts[ns, :L]
    AI = act_ext[:, idx].transpose(1, 0, 2).reshape(k_eff, R)  # (k_eff, R)

    # one-hot actions padded to 128 partitions: oh[a, k, r] = (AI[k, r] == a)
    oh = np.zeros((128, k_eff, R), np.float32)
    kk = np.repeat(np.arange(k_eff), R)
    rr = np.tile(np.arange(R), k_eff)
    oh[AI.reshape(-1), kk, rr] = 1.0
    ohT = oh.astype(BF16)

    # g-gate input-side term, gathered on host: x@W_ih_g.T + b_ih_g
    W_ih = np.asarray(inputs["W_ih"], np.float32)
    b_ih = np.asarray(inputs["b_ih"], np.float32)
    emb_tab = np.asarray(inputs["action_embed"], np.float32)
    GIE_G = np.zeros((NA, 512), np.float32)
    GIE_G[:NLOG] = emb_tab @ W_ih[1024:].T + b_ih[1024:]
    GIE_G[NLOG] = b_ih[1024:]
    gig = GIE_G[AI]                                        # (k_eff, R, 512)
    gigT = np.ascontiguousarray(
        gig.transpose(0, 2, 1).reshape(k_eff, 4, 128, R)).astype(BF16)

    H0 = ro[ns][:, ti]                                     # (NPC, S, H)
    h0T = np.ascontiguousarray(
        H0.transpose(2, 0, 1).reshape(4, 128, R)).astype(BF16)

    ri_ext = np.zeros((NPC, L + K, H), np.float32)
    ri_ext[:, :L] = ri[ns, 1:]
    idx2 = np.asarray(u_list)[:, None] + ti[None, :]       # (F, S)
    TG = ri_ext[:, idx2]                                   # (NPC, F, S, H)
    tgT = np.ascontiguousarray(
        TG.transpose(3, 1, 0, 2).reshape(H, BR).reshape(4, 128, BR)).astype(F8)

    ni = neg_idx.reshape(F, N, S, NEG)[:, ns]              # (F, NPC, S, NEG)
    P = ni.transpose(3, 0, 1, 2).reshape(-1)               # cols in (g, f, j) order
    negs = ri.reshape(N * T, H)[P]
    negsT = np.ascontiguousarray(negs.T.reshape(4, 128, NEG * BR)).astype(F8)

    nd_ext = np.zeros((NPC, L + K), np.float32)
    nd_ext[:, :L] = nd[ns, :L]
    G = nd_ext[:, idx]                                     # (NPC, k_eff, S)
    ndv = G.transpose(1, 0, 2).reshape(k_eff, R)
    ndvT = np.ascontiguousarray(
        ndv.reshape(k_eff, 2, 128).transpose(2, 0, 1)).astype(np.float32)

    return dict(ohT=ohT, gigT=gigT, h0T=h0T, tgT=tgT, negsT=negsT, ndvT=ndvT)


def _prep_weights(inputs):
    W_ih = np.asarray(inputs["W_ih"], np.float32)
    W_hh = np.asarray(inputs["W_hh"], np.float32)
    b_ih = np.asarray(inputs["b_ih"], np.float32)
    b_hh = np.asarray(inputs["b_hh"], np.float32)
    emb_tab = np.asarray(inputs["action_embed"], np.float32)
    W1 = np.asarray(inputs["W1"], np.float32)
    b1 = np.asarray(inputs["b1"], np.float32)
    W2 = np.asarray(inputs["W2"], np.float32)
    b2 = np.asarray(inputs["b2"], np.float32)
    W3 = np.asarray(inputs["W3"], np.float32)
    b3 = np.asarray(inputs["b3"], np.float32)

    d = {}
    # W_hh.T packed for fp8 DoubleRow: [th, ki, ko, 1536]
    d["whh8"] = np.ascontiguousarray(
        W_hh.T.reshape(2, 2, 128, 1536).transpose(0, 2, 1, 3)).astype(F8)
    # gie: action table, padded to 128 rows and chunked contiguously
    # ([128, 12, 128] so each stationary slice is contiguous -> FWL).
    # cols 0:1024 = emb@W_ih_rz.T + b_ih_rz + b_hh_rz (complete r/z
    # input-side term); cols 1024:1536 = b_hh_g broadcast (all rows
    # identical -> the one-hot matmul adds b_hh_g to the g psum).
    gie = np.zeros((128, 1536), np.float32)
    gie[:NLOG, :1024] = emb_tab @ W_ih[:1024].T + b_ih[:1024] + b_hh[:1024]
    gie[NLOG, :1024] = b_ih[:1024] + b_hh[:1024]
    gie[:NA, 1024:] = b_hh[1024:][None, :]
    d["gie"] = np.ascontiguousarray(
        gie.reshape(128, 12, 128)).astype(BF16)

    def pack8(WT):
        # [t, ki, ko, m] with contract index = t*256 + ko*128 + ki
        return np.ascontiguousarray(
            WT.reshape(2, 2, 128, WT.shape[1]).transpose(0, 2, 1, 3)).astype(F8)
    d["w1a8"] = pack8(W1[:, :512].T.copy())
    d["w1b8"] = pack8(W1[:, 512:].T.copy())
    d["w28"] = pack8(W2.T.copy())
    d["w3T"] = np.ascontiguousarray(W3[0].reshape(4, 128).T).astype(BF16)
    d["b1T"] = np.ascontiguousarray(b1.reshape(4, 128).T).astype(np.float32)
    d["b2T"] = np.ascontiguousarray(b2.reshape(4, 128).T).astype(np.float32)
    d["b3f"] = float(b3.reshape(-1)[0])
    return d


# ------------------------------------------------------------- device program

def _gsl(t, c):
    """[128, 2, 512] tile viewed as 4 chunks of [128, 256]."""
    return t[:, c // 2, (c % 2) * 256:(c % 2) * 256 + 256]


def _build_program(u_list, k_eff, b3f):
    nc = bacc.Bacc("TRN2", target_bir_lowering=False, debug=False, num_devices=NC)

    di = {}
    def inp(name, shape, dt):
        di[name] = nc.dram_tensor(name, list(shape), dt, kind="ExternalInput")
        return di[name]

    d_whh = inp("whh8", (2, 128, 2, 1536), DT.float8e4)
    d_gie = inp("gie", (128, 12, 128), DT.bfloat16)
    d_oh = inp("ohT", (128, k_eff, R), DT.bfloat16)
    d_gig = inp("gigT", (k_eff, 4, 128, R), DT.bfloat16)
    d_w1a = inp("w1a8", (2, 128, 2, 512), DT.float8e4)
    d_w1b = inp("w1b8", (2, 128, 2, 512), DT.float8e4)
    d_w2 = inp("w28", (2, 128, 2, 512), DT.float8e4)
    d_w3 = inp("w3T", (128, 4), DT.bfloat16)
    d_b1 = inp("b1T", (128, 4), DT.float32)
    d_b2 = inp("b2T", (128, 4), DT.float32)
    d_h0 = inp("h0T", (4, 128, R), DT.bfloat16)
    d_tg = inp("tgT", (4, 128, BR), DT.float8e4)
    d_negs = inp("negsT", (4, 128, NEG * BR), DT.float8e4)
    d_ndv = inp("ndvT", (128, k_eff, 2), DT.float32)
    d_out = nc.dram_tensor("out", [1, 4], DT.float32, kind="ExternalOutput")

    f32 = DT.float32
    bf16 = DT.bfloat16
    f8 = DT.float8e4

    with tile.TileContext(nc) as tc:
        with (
            tc.tile_pool(name="const", bufs=1) as cp,
            tc.tile_pool(name="gruw", bufs=2) as gp,
            tc.tile_pool(name="mlpw", bufs=3) as mp,
            tc.tile_pool(name="ps", bufs=4, space="PSUM") as pp,
        ):
            # ------------------------------------------------ constant loads
            whh = cp.tile([128, 2, 2, 1536], f8, tag="whh")
            for th in range(2):
                nc.sync.dma_start(out=whh[:, th, :, :], in_=d_whh[th])
            gie = cp.tile([128, 12, 128], bf16, tag="gie")
            nc.sync.dma_start(out=gie[:], in_=d_gie[:])
            oh = cp.tile([128, k_eff, R], bf16, tag="oh")
            nc.sync.dma_start(out=oh[:], in_=d_oh[:])
            w1a = cp.tile([128, 2, 2, 512], f8, tag="w1a")
            w1b = cp.tile([128, 2, 2, 512], f8, tag="w1b")
            w2 = cp.tile([128, 2, 2, 512], f8, tag="w2")
            for (t, d) in ((w1a, d_w1a), (w1b, d_w1b), (w2, d_w2)):
                for th in range(2):
                    nc.sync.dma_start(out=t[:, th, :, :], in_=d[th])
            w3 = cp.tile([128, 4], bf16, tag="w3")
            nc.sync.dma_start(out=w3[:], in_=d_w3[:])
            b1 = cp.tile([128, 4], f32, tag="b1")
            nc.sync.dma_start(out=b1[:], in_=d_b1[:])
            b2 = cp.tile([128, 4], f32, tag="b2")
            nc.sync.dma_start(out=b2[:], in_=d_b2[:])
            tg = cp.tile([128, 4, BR], f8, tag="tg")
            for kc in range(4):
                nc.sync.dma_start(out=tg[:, kc, :], in_=d_tg[kc])
            ndv = cp.tile([128, k_eff, 2], f32, tag="ndv")
            nc.sync.dma_start(out=ndv[:], in_=d_ndv[:])

            # ------------------------------------------------ forward mask
            prod = cp.tile([128, k_eff, 2], f32, tag="prod")
            nc.vector.tensor_scalar(prod[:, 0, :], ndv[:, 0, :], 0.0, None,
                                    op0=ALU.is_gt)
            for k in range(1, k_eff):
                nc.vector.scalar_tensor_tensor(
                    prod[:, k, :], in0=ndv[:, k, :], scalar=0.0,
                    in1=prod[:, k - 1, :], op0=ALU.is_gt, op1=ALU.mult)
            mfT = cp.tile([128, 2 * F], f32, tag="mfT")
            for fi, u in enumerate(u_list):
                nc.vector.tensor_copy(mfT[:, 2 * fi:2 * fi + 2], prod[:, u, :])

            # ------------------------------------------------ GRU (bf16)
            # Gate order in the 1536 dim: r(512), z(512), g(512).
            # Per step: 48 W_hh matmuls + 12 one-hot matmuls (gi / biases
            # folded into PSUM).  r/z sigmoid straight from PSUM on the
            # scalar engine; g tail per 128-H-chunk so the next step's
            # matmuls start as soon as chunk 0 of h_new is written.
            h_prev = gp.tile([128, 4, R], bf16, tag="h")
            for kc in range(4):
                nc.sync.dma_start(out=h_prev[:, kc, :], in_=d_h0[kc])
            h8_prev = gp.tile([128, 4, R], f8, tag="h8")
            nc.vector.tensor_copy(h8_prev[:], h_prev[:])
            predsT = cp.tile([128, 4, BR], f8, tag="preds")
            DRM = mybir.MatmulPerfMode.DoubleRow

            for k in range(k_eff):
                gig = gp.tile([128, 4, R], bf16, tag="gig", bufs=3)
                for kc in range(4):
                    nc.sync.dma_start(out=gig[:, kc, :], in_=d_gig[k, kc])

                ps_r = pp.tile([128, 2, 512], f32, tag="ps")
                ps_g = pp.tile([128, 2, 512], f32, tag="ps")
                ps_z = pp.tile([128, 2, 512], f32, tag="ps")
                r_sb = gp.tile([128, 4, R], bf16, tag="r")
                z_sb = gp.tile([128, 4, R], bf16, tag="z")
                t_sb = gp.tile([128, 4, R], bf16, tag="t")
                u_sb = gp.tile([128, 4, R], bf16, tag="u")
                g_sb = gp.tile([128, 4, R], bf16, tag="g")
                d_sb = gp.tile([128, 4, R], bf16, tag="d")
                hz_sb = gp.tile([128, 4, R], bf16, tag="hz")
                h_new = gp.tile([128, 4, R], bf16, tag="h")
                h8_new = gp.tile([128, 4, R], f8, tag="h8")

                # psum region for gate chunk: r = ps_r c0..3, z = ps_z,
                # g = ps_g.  MM emission is 3-phase so the tensor queue never
                # stalls: one-hot matmuls first (no h dependency -> they run
                # during the previous step's tail), then all th=0 DR matmuls
                # (need only h8 pair 0), then th=1 ordered r, g, z.
                def reg(gc):
                    return _gsl((ps_r, ps_z, ps_g)[gc // 4], gc % 4)

                for gc in range(12):
                    nc.tensor.matmul(reg(gc), gie[:, gc, :], oh[:, k, :],
                                     start=True, stop=False)
                for order in ((0, 1, 2), (0, 2, 1)):
                    th = 0 if order == (0, 1, 2) else 1
                    for grp in order:
                        for c in range(4):
                            gc = grp * 4 + c
                            sl = slice(gc * 128, (gc + 1) * 128)
                            nc.tensor.matmul(
                                reg(gc), whh[:, th, :, sl],
                                h8_prev[:, 2 * th:2 * th + 2, :],
                                start=False, stop=(th == 1), perf_mode=DRM)
                for c in range(4):
                    nc.scalar.activation(r_sb[:, c, :], _gsl(ps_r, c),
                                         AF.Sigmoid)
                # tail: chunks 0,1 fast-tracked (they gate the next step's
                # th=0 matmuls via the pair-0 f8 cast); chunks 2,3 follow.
                def tail_head(c):
                    nc.vector.tensor_mul(t_sb[:, c, :], _gsl(ps_g, c),
                                         r_sb[:, c, :])
                    nc.vector.tensor_add(u_sb[:, c, :], gig[:, c, :],
                                         t_sb[:, c, :])
                    nc.scalar.activation(g_sb[:, c, :], u_sb[:, c, :], AF.Tanh)
                    nc.scalar.activation(z_sb[:, c, :], _gsl(ps_z, c),
                                         AF.Sigmoid)
                def tail_rest(c):
                    nc.vector.tensor_sub(d_sb[:, c, :], h_prev[:, c, :],
                                         g_sb[:, c, :])
                    nc.vector.tensor_mul(hz_sb[:, c, :], z_sb[:, c, :],
                                         d_sb[:, c, :])
                    nc.vector.tensor_add(h_new[:, c, :], g_sb[:, c, :],
                                         hz_sb[:, c, :])
                tail_head(0)
                tail_head(1)
                tail_rest(0)
                tail_rest(1)
                nc.vector.tensor_copy(h8_new[:, 0:2, :], h_new[:, 0:2, :])
                tail_head(2)
                tail_head(3)
                tail_rest(2)
                tail_rest(3)
                nc.scalar.activation(h8_new[:, 2:4, :], h_new[:, 2:4, :],
                                     AF.Copy)
                h_prev = h_new
                h8_prev = h8_new
                for fi, u in enumerate(u_list):
                    if u == k:
                        nc.vector.tensor_copy(
                            predsT[:, :, fi * R:(fi + 1) * R], h_new[:])

            # ------------------------------------------------ p1 = preds@W1a + b1
            DR = mybir.MatmulPerfMode.DoubleRow
            p1e = cp.tile([128, 4, BR], bf16, tag="p1e")
            for cc in range(4):
                ps = pp.tile([128, 2, 512], f32, tag="ps")
                for rt in range(2):
                    sl = slice(rt * 512, (rt + 1) * 512)
                    for th in range(2):
                        nc.tensor.matmul(
                            ps[:, rt, :],
                            w1a[:, th, :, cc * 128:(cc + 1) * 128],
                            predsT[:, 2 * th:2 * th + 2, sl],
                            start=(th == 0), stop=(th == 1), perf_mode=DR)
                nc.scalar.activation(p1e[:, cc, :], ps[:], AF.Identity,
                                     bias=b1[:, cc:cc + 1])

            # ------------------------------------------------ blocks
            # L1: only x@W1b on the tensor engine; eviction adds p1e
            # (vector STT) and applies ReLU (scalar for cc 0/1, vector for
            # cc 2/3 to balance the two engines).
            logits = cp.tile([128, NBLK, 8], f32, tag="logits")
            y2_pend = []
            for b in range(NBLK):
                if b < NEG:
                    xt = mp.tile([128, 4, BR], f8, tag="negsx")
                    for kc in range(4):
                        nc.sync.dma_start(
                            out=xt[:, kc, :],
                            in_=d_negs[kc][:, b * BR:(b + 1) * BR])
                else:
                    xt = tg
                y1 = mp.tile([128, 4, BR], f8, tag="y1", bufs=2)
                y1p = mp.tile([128, 4, BR], bf16, tag="y1p", bufs=2)
                for cc in range(4):
                    ps = pp.tile([128, 2, 512], f32, tag="ps")
                    for rt in range(2):
                        sl = slice(rt * 512, (rt + 1) * 512)
                        for th in range(2):
                            nc.tensor.matmul(
                                ps[:, rt, :],
                                w1b[:, th, :, cc * 128:(cc + 1) * 128],
                                xt[:, 2 * th:2 * th + 2, sl],
                                start=(th == 0), stop=(th == 1), perf_mode=DR)
                    nc.vector.scalar_tensor_tensor(
                        y1p[:, cc, :], in0=ps[:], scalar=0.0,
                        in1=p1e[:, cc, :], op0=ALU.add, op1=ALU.add)
                    if cc < 2:
                        nc.scalar.activation(y1[:, cc, :], y1p[:, cc, :],
                                             AF.Relu)
                    else:
                        nc.vector.tensor_scalar(y1[:, cc, :], y1p[:, cc, :],
                                                0.0, None, op0=ALU.max)
                y2 = mp.tile([128, 4, BR], bf16, tag="y2", bufs=2)
                for cc in range(4):
                    ps = pp.tile([128, 2, 512], f32, tag="ps")
                    for rt in range(2):
                        sl = slice(rt * 512, (rt + 1) * 512)
                        for th in range(2):
                            nc.tensor.matmul(
                                ps[:, rt, :],
                                w2[:, th, :, cc * 128:(cc + 1) * 128],
                                y1[:, 2 * th:2 * th + 2, sl],
                                start=(th == 0), stop=(th == 1), perf_mode=DR)
                    nc.scalar.activation(y2[:, cc, :], ps[:], AF.Relu,
                                         bias=b2[:, cc:cc + 1])
                # L3 for this block is emitted at the top of the next block
                # (after its L1 matmuls) so the in-order tensor queue doesn't
                # stall on the y2 ReLU chain.
                y2_pend.append((b, y2))
                while len(y2_pend) > (1 if b < NBLK - 1 else 0):
                    bb, yy2 = y2_pend.pop(0)
                    ps3 = pp.tile([128, 2, 512], f32, tag="ps")
                    for col in range(8):
                        for kc in range(4):
                            nc.tensor.matmul(
                                ps3[:, 0, col:col + 1],
                                yy2[:, kc, col * 128:(col + 1) * 128],
                                w3[:, kc:kc + 1], start=(kc == 0),
                                stop=(kc == 3))
                    nc.scalar.activation(logits[:, bb, :], ps3[:, 0, 0:8],
                                         AF.Copy)

            # ------------------------------------- softplus + sums
            # softplus(t) = relu(t) - ln(sigmoid(|t|)); whole-tensor ACT ops
            # keep the activation-table sequence to a single switch.
            partials = cp.tile([128, NBLK + 1], f32, tag="partials")
            sp_a = cp.tile([128, NBLK, 8], f32, tag="sp_a")
            sp_l = cp.tile([128, NBLK, 8], f32, tag="sp_l")
            sp_r = cp.tile([128, NBLK, 8], f32, tag="sp_r")
            sp_d = cp.tile([128, 8], f32, tag="sp_d")
            nc.scalar.activation(sp_a[:], logits[:], AF.Abs, bias=b3f)
            nc.scalar.activation(sp_a[:], sp_a[:], AF.Sigmoid)
            nc.scalar.activation(sp_l[:], sp_a[:], AF.Ln)
            nc.scalar.activation(sp_r[:, :NEG, :], logits[:, :NEG, :],
                                 AF.Relu, bias=b3f)
            nc.scalar.activation(sp_r[:, NEG, :], logits[:, NEG, :],
                                 AF.Relu, bias=-b3f, scale=-1.0)
            nc.vector.tensor_sub(sp_r[:], sp_r[:], sp_l[:])
            for b in range(NBLK):
                nc.vector.tensor_mul(sp_d[:], sp_r[:, b, :], mfT[:])
                nc.vector.tensor_reduce(partials[:, b:b + 1], sp_d[:],
                                        mybir.AxisListType.X, ALU.add)
            nc.vector.tensor_reduce(partials[:, NBLK:NBLK + 1], mfT[:],
                                    mybir.AxisListType.X, ALU.add)

            vcol = cp.tile([128, 4], f32, tag="vcol")
            nc.vector.tensor_copy(vcol[:, 0:1], partials[:, NEG:NEG + 1])
            nc.vector.tensor_reduce(vcol[:, 1:2], partials[:, 0:NEG],
                                    mybir.AxisListType.X, ALU.add)
            nc.vector.tensor_copy(vcol[:, 2:3], partials[:, NBLK:NBLK + 1])
            nc.any.memset(vcol[:, 3:4], 0.0)
            ones = cp.tile([128, 1], f32, tag="ones")
            nc.any.memset(ones[:], 1.0)
            psf = pp.tile([128, 2, 512], f32, tag="ps")
            nc.tensor.matmul(psf[0:1, 0, 0:4], ones[:], vcol[:],
                             start=True, stop=True)
            out_sb = cp.tile([1, 4], f32, tag="out_sb")
            nc.scalar.activation(out_sb[:], psf[0:1, 0, 0:4], AF.Copy)
            nc.sync.dma_start(out=d_out[:], in_=out_sb[:])

    nc.finalize()
    return nc


def _get_program(u_list, k_eff, b3f):
    key = (tuple(u_list), k_eff, float(b3f))
    if key not in _PROGRAM_CACHE:
        _PROGRAM_CACHE[key] = _build_program(u_list, k_eff, b3f)
    return _PROGRAM_CACHE[key]


# ------------------------------------------------------------------ kernel

def kernel(**inputs):
    u_list = [int(x) for x in np.asarray(inputs["unroll_subsample"]).reshape(-1)]
    k_eff = max(u_list) + 1
    w = _prep_weights(inputs)
    nc = _get_program(u_list, k_eff, w["b3f"])

    wmaps = {k: v for k, v in w.items() if k != "b3f"}
    in_maps = []
    for c in range(NC):
        m = dict(wmaps)
        m.update(_prep_core(c, inputs, u_list, k_eff))
        in_maps.append(m)

    res = bass_utils.run_bass_kernel_spmd(nc, in_maps, list(range(NC)))
    P = Ng = D = 0.0
    for c in range(NC):
        o = np.asarray(res.results[c]["out"], np.float64)
        P += o[0, 0]
        Ng += o[0, 1]
        D += o[0, 2]
    loss = COEFF * (P / D + Ng / (D * NEG))
    return np.float32(loss)


# revision 7
# speedup vs baseline: 1.5920x; 1.1845x over previous
"""Trainium2 Bass kernel for the CPCA auxiliary loss (nn_CPCA_51754355917033).

Strategy (data-parallel over the env/batch dim n, 16 envs per core):
  - GRU runs fully in bf16.  The input-side gate terms (x@W_ih.T + biases)
    are folded into the same PSUM accumulation as W_hh@h via one-hot action
    matmuls against a 19-row table (emb@W_ih.T + b_ih [+ b_hh]), so the only
    per-step vector work is the g-gate tail; r/z evict straight from PSUM
    through the scalar engine's sigmoid.  The gate tail is split into four
    128-row H-chunks so the next step's matmuls (which contract one H-chunk
    each) start as soon as the first chunk of h is ready.
  - MLP: preds@W1a + b1 is hoisted out of the 21-block loop (computed once,
    stashed as bf16 "p1e"); per block only x@W1b runs on the tensor engine
    and the eviction adds p1e (vector) + ReLU (scalar/vector split).
  - Host combines the 8 cores' (pos_sum, neg_sum, denom) partials into the
    final scalar (the all-reduce of the sharding hint, done at unshard time).
"""

import numpy as np
import ml_dtypes

import concourse.bass as bass
import concourse.mybir as mybir
import concourse.tile as tile
from concourse import bacc
from concourse import bass_utils

BF16 = ml_dtypes.bfloat16
F8 = ml_dtypes.float8_e4m3
DT = mybir.dt
AF = mybir.ActivationFunctionType
ALU = mybir.AluOpType

N, T, H, K, S, F, EMB, NLOG, NEG = 128, 512, 512, 16, 16, 4, 32, 18, 20
COEFF = 0.1
NC = 8
NPC = N // NC          # envs per core
R = NPC * S            # GRU rows per core (256)
L = T - 1
NBLK = NEG + 1         # 20 negative g-blocks + 1 positive block
BR = F * R             # rows per block (1024)
NA = NLOG + 1          # action vocab + padding row

_PROGRAM_CACHE = {}


# ----------------------------------------------------------------- host prep

def _prep_core(c, inputs, u_list, k_eff):
    acts = np.asarray(inputs["actions"])[..., 0]
    nd = np.asarray(inputs["not_dones"])[..., 0]
    ri = np.asarray(inputs["rnn_inputs"], np.float32)
    ro = np.asarray(inputs["rnn_outputs"], np.float32)
    ti = np.asarray(inputs["time_subsample"]).astype(np.int64)
    neg_idx = np.asarray(inputs["neg_idx"]).astype(np.int64)

    ns = slice(c * NPC, (c + 1) * NPC)
    idx = np.arange(k_eff)[:, None] + ti[None, :]          # (k_eff, S)

    act_ext = np.full((NPC, L + K), NLOG, np.int64)
    act_ext[:, :L] = acts[ns, :L]
    AI = act_ext[:, idx].transpose(1, 0, 2).reshape(k_eff, R)  # (k_eff, R)

    # one-hot actions padded to 128 partitions: oh[a, k, r] = (AI[k, r] == a)
    oh = np.zeros((128, k_eff, R), np.float32)
    kk = np.repeat(np.arange(k_eff), R)
    rr = np.tile(np.arange(R), k_eff)
    oh[AI.reshape(-1), kk, rr] = 1.0
    ohT = oh.astype(BF16)

    # g-gate input-side term, gathered on host: x@W_ih_g.T + b_ih_g
    W_ih = np.asarray(inputs["W_ih"], np.float32)
    b_ih = np.asarray(inputs["b_ih"], np.float32)
    emb_tab = np.asarray(inputs["action_embed"], np.float32)
    GIE_G = np.zeros((NA, 512), np.float32)
    GIE_G[:NLOG] = emb_tab @ W_ih[1024:].T + b_ih[1024:]
    GIE_G[NLOG] = b_ih[1024:]
    gig = GIE_G[AI]                                        # (k_eff, R, 512)
    gigT = np.ascontiguousarray(
        gig.transpose(0, 2, 1).reshape(k_eff, 4, 128, R)).astype(BF16)

    H0 = ro[ns][:, ti]                                     # (NPC, S, H)
    h0T = np.ascontiguousarray(
        H0.transpose(2, 0, 1).reshape(4, 128, R)).astype(BF16)

    ri_ext = np.zeros((NPC, L + K, H), np.float32)
    ri_ext[:, :L] = ri[ns, 1:]
    idx2 = np.asarray(u_list)[:, None] + ti[None, :]       # (F, S)
    TG = ri_ext[:, idx2]                                   # (NPC, F, S, H)
    tgT = np.ascontiguousarray(
        TG.transpose(3, 1, 0, 2).reshape(H, BR).reshape(4, 128, BR)).astype(F8)

    ni = neg_idx.reshape(F, N, S, NEG)[:, ns]              # (F, NPC, S, NEG)
    P = ni.transpose(3, 0, 1, 2).reshape(-1)               # cols in (g, f, j) order
    negs = ri.reshape(N * T, H)[P]
    negsT = np.ascontiguousarray(negs.T.reshape(4, 128, NEG * BR)).astype(F8)

    nd_ext = np.zeros((NPC, L + K), np.float32)
    nd_ext[:, :L] = nd[ns, :L]
    G = nd_ext[:, idx]                                     # (NPC, k_eff, S)
    ndv = G.transpose(1, 0, 2).reshape(k_eff, R)
    ndvT = np.ascontiguousarray(
        ndv.reshape(k_eff, 2, 128).transpose(2, 0, 1)).astype(np.float32)

    return dict(ohT=ohT, gigT=gigT, h0T=h0T, tgT=tgT, negsT=negsT, ndvT=ndvT)


def _prep_weights(inputs):
    W_ih = np.asarray(inputs["W_ih"], np.float32)
    W_hh = np.asarray(inputs["W_hh"], np.float32)
    b_ih = np.asarray(inputs["b_ih"], np.float32)
    b_hh = np.asarray(inputs["b_hh"], np.float32)
    emb_tab = np.asarray(inputs["action_embed"], np.float32)
    W1 = np.asarray(inputs["W1"], np.float32)
    b1 = np.asarray(inputs["b1"], np.float32)
    W2 = np.asarray(inputs["W2"], np.float32)
    b2 = np.asarray(inputs["b2"], np.float32)
    W3 = np.asarray(inputs["W3"], np.float32)
    b3 = np.asarray(inputs["b3"], np.float32)

    d = {}
    # W_hh.T packed for fp8 DoubleRow: [th, ki, ko, 1536]
    d["whh8"] = np.ascontiguousarray(
        W_hh.T.reshape(2, 2, 128, 1536).transpose(0, 2, 1, 3)).astype(F8)
    # gie: action table, padded to 128 rows and chunked contiguously
    # ([128, 12, 128] so each stationary slice is contiguous -> FWL).
    # cols 0:1024 = emb@W_ih_rz.T + b_ih_rz + b_hh_rz (complete r/z
    # input-side term); cols 1024:1536 = b_hh_g broadcast (all rows
    # identical -> the one-hot matmul adds b_hh_g to the g psum).
    gie = np.zeros((128, 1536), np.float32)
    gie[:NLOG, :1024] = emb_tab @ W_ih[:1024].T + b_ih[:1024] + b_hh[:1024]
    gie[NLOG, :1024] = b_ih[:1024] + b_hh[:1024]
    gie[:NA, 1024:] = b_hh[1024:][None, :]
    d["gie"] = np.ascontiguousarray(
        gie.reshape(128, 12, 128)).astype(BF16)

    def pack8(WT):
        # [t, ki, ko, m] with contract index = t*256 + ko*128 + ki
        return np.ascontiguousarray(
            WT.reshape(2, 2, 128, WT.shape[1]).transpose(0, 2, 1, 3)).astype(F8)
    d["w1a8"] = pack8(W1[:, :512].T.copy())
    d["w1b8"] = pack8(W1[:, 512:].T.copy())
    d["w28"] = pack8(W2.T.copy())
    d["w3T"] = np.ascontiguousarray(W3[0].reshape(4, 128).T).astype(BF16)
    d["b1T"] = np.ascontiguousarray(b1.reshape(4, 128).T).astype(np.float32)
    d["b2T"] = np.ascontiguousarray(b2.reshape(4, 128).T).astype(np.float32)
    d["b3f"] = float(b3.reshape(-1)[0])
    return d


# ------------------------------------------------------------- device program

def _gsl(t, c):
    """[128, 2, 512] tile viewed as 4 chunks of [128, 256]."""
    return t[:, c // 2, (c % 2) * 256:(c % 2) * 256 + 256]


def _build_program(u_list, k_eff, b3f):
    nc = bacc.Bacc("TRN2", target_bir_lowering=False, debug=False, num_devices=NC)

    di = {}
    def inp(name, shape, dt):
        di[name] = nc.dram_tensor(name, list(shape), dt, kind="ExternalInput")
        return di[name]

    d_whh = inp("whh8", (2, 128, 2, 1536), DT.float8e4)
    d_gie = inp("gie", (128, 12, 128), DT.bfloat16)
    d_oh = inp("ohT", (128, k_eff, R), DT.bfloat16)
    d_gig = inp("gigT", (k_eff, 4, 128, R), DT.bfloat16)
    d_w1a = inp("w1a8", (2, 128, 2, 512), DT.float8e4)
    d_w1b = inp("w1b8", (2, 128, 2, 512), DT.float8e4)
    d_w2 = inp("w28", (2, 128, 2, 512), DT.float8e4)
    d_w3 = inp("w3T", (128, 4), DT.bfloat16)
    d_b1 = inp("b1T", (128, 4), DT.float32)
    d_b2 = inp("b2T", (128, 4), DT.float32)
    d_h0 = inp("h0T", (4, 128, R), DT.bfloat16)
    d_tg = inp("tgT", (4, 128, BR), DT.float8e4)
    d_negs = inp("negsT", (4, 128, NEG * BR), DT.float8e4)
    d_ndv = inp("ndvT", (128, k_eff, 2), DT.float32)
    d_out = nc.dram_tensor("out", [1, 4], DT.float32, kind="ExternalOutput")

    f32 = DT.float32
    bf16 = DT.bfloat16
    f8 = DT.float8e4

    with tile.TileContext(nc) as tc:
        with (
            tc.tile_pool(name="const", bufs=1) as cp,
            tc.tile_pool(name="gruw", bufs=2) as gp,
            tc.tile_pool(name="mlpw", bufs=3) as mp,
            tc.tile_pool(name="ps", bufs=4, space="PSUM") as pp,
        ):
            # ------------------------------------------------ constant loads
            whh = cp.tile([128, 2, 2, 1536], f8, tag="whh")
            for th in range(2):
                nc.sync.dma_start(out=whh[:, th, :, :], in_=d_whh[th])
            gie = cp.tile([128, 12, 128], bf16, tag="gie")
            nc.sync.dma_start(out=gie[:], in_=d_gie[:])
            oh = cp.tile([128, k_eff, R], bf16, tag="oh")
            nc.sync.dma_start(out=oh[:], in_=d_oh[:])
            w1a = cp.tile([128, 2, 2, 512], f8, tag="w1a")
            w1b = cp.tile([128, 2, 2, 512], f8, tag="w1b")
            w2 = cp.tile([128, 2, 2, 512], f8, tag="w2")
            for (t, d) in ((w1a, d_w1a), (w1b, d_w1b), (w2, d_w2)):
                for th in range(2):
                    nc.sync.dma_start(out=t[:, th, :, :], in_=d[th])
            w3 = cp.tile([128, 4], bf16, tag="w3")
            nc.sync.dma_start(out=w3[:], in_=d_w3[:])
            b1 = cp.tile([128, 4], f32, tag="b1")
            nc.sync.dma_start(out=b1[:], in_=d_b1[:])
            b2 = cp.tile([128, 4], f32, tag="b2")
            nc.sync.dma_start(out=b2[:], in_=d_b2[:])
            tg = cp.tile([128, 4, BR], f8, tag="tg")
            for kc in range(4):
                nc.sync.dma_start(out=tg[:, kc, :], in_=d_tg[kc])
            ndv = cp.tile([128, k_eff, 2], f32, tag="ndv")
            nc.sync.dma_start(out=ndv[:], in_=d_ndv[:])

            # ------------------------------------------------ forward mask
            prod = cp.tile([128, k_eff, 2], f32, tag="prod")
            nc.vector.tensor_scalar(prod[:, 0, :], ndv[:, 0, :], 0.0, None,
                                    op0=ALU.is_gt)
            for k in range(1, k_eff):
                nc.vector.scalar_tensor_tensor(
                    prod[:, k, :], in0=ndv[:, k, :], scalar=0.0,
                    in1=prod[:, k - 1, :], op0=ALU.is_gt, op1=ALU.mult)
            mfT = cp.tile([128, 2 * F], f32, tag="mfT")
            for fi, u in enumerate(u_list):
                nc.vector.tensor_copy(mfT[:, 2 * fi:2 * fi + 2], prod[:, u, :])

            # ------------------------------------------------ GRU (bf16)
            # Gate order in the 1536 dim: r(512), z(512), g(512).
            # Per step: 48 W_hh matmuls + 12 one-hot matmuls (gi / biases
            # folded into PSUM).  r/z sigmoid straight from PSUM on the
            # scalar engine; g tail per 128-H-chunk so the next step's
            # matmuls start as soon as chunk 0 of h_new is written.
            h_prev = gp.tile([128, 4, R], bf16, tag="h")
            for kc in range(4):
                nc.sync.dma_start(out=h_prev[:, kc, :], in_=d_h0[kc])
            h8_prev = gp.tile([128, 4, R], f8, tag="h8")
            nc.vector.tensor_copy(h8_prev[:], h_prev[:])
            predsT = cp.tile([128, 4, BR], f8, tag="preds")
            DRM = mybir.MatmulPerfMode.DoubleRow

            for k in range(k_eff):
                gig = gp.tile([128, 4, R], bf16, tag="gig", bufs=3)
                for kc in range(4):
                    nc.sync.dma_start(out=gig[:, kc, :], in_=d_gig[k, kc])

                ps_r = pp.tile([128, 2, 512], f32, tag="ps")
                ps_g = pp.tile([128, 2, 512], f32, tag="ps")
                ps_z = pp.tile([128, 2, 512], f32, tag="ps")
                r_sb = gp.tile([128, 4, R], bf16, tag="r")
                z_sb = gp.tile([128, 4, R], bf16, tag="z")
                t_sb = gp.tile([128, 4, R], bf16, tag="t")
                u_sb = gp.tile([128, 4, R], bf16, tag="u")
                g_sb = gp.tile([128, 4, R], bf16, tag="g")
                d_sb = gp.tile([128, 4, R], bf16, tag="d")
                hz_sb = gp.tile([128, 4, R], bf16, tag="hz")
                h_new = gp.tile([128, 4, R], bf16, tag="h")
                h8_new = gp.tile([128, 4, R], f8, tag="h8")

                # psum region for gate chunk: r = ps_r c0..3, z = ps_z,
                # g = ps_g.  MM emission is 3-phase so the tensor queue never
                # stalls: one-hot matmuls first (no h dependency -> they run
                # during the previous step's tail), then all th=0 DR matmuls
                # (need only h8 pair 0), then th=1 ordered r, g, z.
                def reg(gc):
                    return _gsl((ps_r, ps_z, ps_g)[gc // 4], gc % 4)

                for gc in range(12):
                    nc.tensor.matmul(reg(gc), gie[:, gc, :], oh[:, k, :],
                                     start=True, stop=False)
                for order in ((0, 1, 2), (0, 2, 1)):
                    th = 0 if order == (0, 1, 2) else 1
                    for grp in order:
                        for c in range(4):
                            gc = grp * 4 + c
                            sl = slice(gc * 128, (gc + 1) * 128)
                            nc.tensor.matmul(
                                reg(gc), whh[:, th, :, sl],
                                h8_prev[:, 2 * th:2 * th + 2, :],
                                start=False, stop=(th == 1), perf_mode=DRM)
                # tail in half-batches (chunk pairs).  The f8 h8_new pair
                # is written directly by the final add, so the next step's
                # th=0 matmuls start as soon as pair 0 lands; the bf16 copy
                # of h (needed for the next step's d = h - g) trails lazily.
                def half(tl, p):
                    return tl[:, 2 * p:2 * p + 2, :]
                def tail(p):
                    nc.scalar.activation(half(r_sb, p), ps_r[:, p, :],
                                         AF.Sigmoid)
                    nc.vector.tensor_mul(half(t_sb, p), ps_g[:, p, :],
                                         half(r_sb, p))
                    nc.vector.tensor_add(half(u_sb, p), half(gig, p),
                                         half(t_sb, p))
                    nc.scalar.activation(half(g_sb, p), half(u_sb, p),
                                         AF.Tanh)
                    nc.scalar.activation(half(z_sb, p), ps_z[:, p, :],
                                         AF.Sigmoid)
                    nc.vector.tensor_sub(half(d_sb, p), half(h_prev, p),
                                         half(g_sb, p))
                    nc.vector.tensor_mul(half(hz_sb, p), half(z_sb, p),
                                         half(d_sb, p))
                    nc.vector.tensor_add(half(h8_new, p), half(g_sb, p),
                                         half(hz_sb, p))
                tail(0)
                tail(1)
                nc.vector.tensor_add(h_new[:, 0:2, :], g_sb[:, 0:2, :],
                                     hz_sb[:, 0:2, :])
                nc.vector.tensor_add(h_new[:, 2:4, :], g_sb[:, 2:4, :],
                                     hz_sb[:, 2:4, :])
                h_prev = h_new
                h8_prev = h8_new
                for fi, u in enumerate(u_list):
                    if u == k:
                        nc.vector.tensor_copy(
                            predsT[:, :, fi * R:(fi + 1) * R], h_new[:])

            # ------------------------------------------------ p1 = preds@W1a + b1
            DR = mybir.MatmulPerfMode.DoubleRow
            p1e = cp.tile([128, 4, BR], bf16, tag="p1e")
            for cc in range(4):
                ps = pp.tile([128, 2, 512], f32, tag="ps")
                for rt in range(2):
                    sl = slice(rt * 512, (rt + 1) * 512)
                    for th in range(2):
                        nc.tensor.matmul(
                            ps[:, rt, :],
                            w1a[:, th, :, cc * 128:(cc + 1) * 128],
                            predsT[:, 2 * th:2 * th + 2, sl],
                            start=(th == 0), stop=(th == 1), perf_mode=DR)
                nc.scalar.activation(p1e[:, cc, :], ps[:], AF.Identity,
                                     bias=b1[:, cc:cc + 1])

            # ------------------------------------------------ blocks
            # Software-pipelined: iteration b emits L1(b), L2(b-1), L3(b-2)
            # so the in-order tensor queue never waits on an eviction chain.
            # L1 eviction adds the hoisted p1e (vector STT) + ReLU (scalar
            # for chunks 0/1, vector for 2/3).  Softplus for the 20 negative
            # blocks is emitted as soon as their logits exist; only the
            # positive block's softplus trails the last matmul.
            logits = cp.tile([128, NBLK, 8], f32, tag="logits")
            partials = cp.tile([128, NBLK + 1], f32, tag="partials")
            sp_a = cp.tile([128, NBLK, 8], f32, tag="sp_a")
            sp_l = cp.tile([128, NBLK, 8], f32, tag="sp_l")
            sp_r = cp.tile([128, NBLK, 8], f32, tag="sp_r")
            sp_d = cp.tile([128, 8], f32, tag="sp_d")
            nc.vector.tensor_reduce(partials[:, NBLK:NBLK + 1], mfT[:],
                                    mybir.AxisListType.X, ALU.add)
            l2_pend = []
            l3_pend = []
            for b in range(NBLK + 2):
                if b < NBLK:
                    if b < NEG:
                        xt = mp.tile([128, 4, BR], f8, tag="negsx")
                        for kc in range(4):
                            nc.sync.dma_start(
                                out=xt[:, kc, :],
                                in_=d_negs[kc][:, b * BR:(b + 1) * BR])
                    else:
                        xt = tg
                    y1 = mp.tile([128, 4, BR], f8, tag="y1", bufs=2)
                    y1p = mp.tile([128, 4, BR], bf16, tag="y1p", bufs=2)
                    for cc in range(4):
                        ps = pp.tile([128, 2, 512], f32, tag="ps")
                        for rt in range(2):
                            sl = slice(rt * 512, (rt + 1) * 512)
                            for th in range(2):
                                nc.tensor.matmul(
                                    ps[:, rt, :],
                                    w1b[:, th, :, cc * 128:(cc + 1) * 128],
                                    xt[:, 2 * th:2 * th + 2, sl],
                                    start=(th == 0), stop=(th == 1),
                                    perf_mode=DR)
                        nc.vector.scalar_tensor_tensor(
                            y1p[:, cc, :], in0=ps[:], scalar=0.0,
                            in1=p1e[:, cc, :], op0=ALU.add, op1=ALU.add)
                        if cc < 2:
                            nc.scalar.activation(y1[:, cc, :], y1p[:, cc, :],
                                                 AF.Relu)
                        else:
                            nc.vector.tensor_scalar(y1[:, cc, :],
                                                    y1p[:, cc, :],
                                                    0.0, None, op0=ALU.max)
                    l2_pend.append(y1)
                if b >= 1 and l2_pend:
                    yy1 = l2_pend.pop(0)
                    y2 = mp.tile([128, 4, BR], bf16, tag="y2", bufs=2)
                    for cc in range(4):
                        ps = pp.tile([128, 2, 512], f32, tag="ps")
                        for rt in range(2):
                            sl = slice(rt * 512, (rt + 1) * 512)
                            for th in range(2):
                                nc.tensor.matmul(
                                    ps[:, rt, :],
                                    w2[:, th, :, cc * 128:(cc + 1) * 128],
                                    yy1[:, 2 * th:2 * th + 2, sl],
                                    start=(th == 0), stop=(th == 1),
                                    perf_mode=DR)
                        nc.scalar.activation(y2[:, cc, :], ps[:], AF.Relu,
                                             bias=b2[:, cc:cc + 1])
                    l3_pend.append(y2)
                if b >= 2 and l3_pend:
                    bb = b - 2
                    yy2 = l3_pend.pop(0)
                    ps3 = pp.tile([128, 2, 512], f32, tag="ps")
                    for col in range(8):
                        for kc in range(4):
                            nc.tensor.matmul(
                                ps3[:, 0, col:col + 1],
                                yy2[:, kc, col * 128:(col + 1) * 128],
                                w3[:, kc:kc + 1], start=(kc == 0),
                                stop=(kc == 3))
                    nc.scalar.activation(logits[:, bb, :], ps3[:, 0, 0:8],
                                         AF.Copy)
                    if bb == NEG - 1:
                        # softplus(t) = relu(t) - ln(sigmoid(|t|)), negatives
                        nc.scalar.activation(sp_a[:, :NEG, :],
                                             logits[:, :NEG, :],
                                             AF.Abs, bias=b3f)
                        nc.scalar.activation(sp_a[:, :NEG, :],
                                             sp_a[:, :NEG, :], AF.Sigmoid)
                        nc.scalar.activation(sp_l[:, :NEG, :],
                                             sp_a[:, :NEG, :], AF.Ln)
                        nc.scalar.activation(sp_r[:, :NEG, :],
                                             logits[:, :NEG, :],
                                             AF.Relu, bias=b3f)
                        nc.vector.tensor_sub(sp_r[:, :NEG, :],
                                             sp_r[:, :NEG, :],
                                             sp_l[:, :NEG, :])
                        for q in range(NEG):
                            nc.vector.tensor_mul(sp_d[:], sp_r[:, q, :],
                                                 mfT[:])
                            nc.vector.tensor_reduce(partials[:, q:q + 1],
                                                    sp_d[:],
                                                    mybir.AxisListType.X,
                                                    ALU.add)

            # positive block softplus (BCE vs 1 -> softplus(-logit))
            nc.scalar.activation(sp_a[:, NEG, :], logits[:, NEG, :],
                                 AF.Abs, bias=b3f)
            nc.scalar.activation(sp_a[:, NEG, :], sp_a[:, NEG, :], AF.Sigmoid)
            nc.scalar.activation(sp_l[:, NEG, :], sp_a[:, NEG, :], AF.Ln)
            nc.scalar.activation(sp_r[:, NEG, :], logits[:, NEG, :],
                                 AF.Relu, bias=-b3f, scale=-1.0)
            nc.vector.tensor_sub(sp_r[:, NEG, :], sp_r[:, NEG, :],
                                 sp_l[:, NEG, :])
            nc.vector.tensor_mul(sp_d[:], sp_r[:, NEG, :], mfT[:])
            nc.vector.tensor_reduce(partials[:, NEG:NEG + 1], sp_d[:],
                                    mybir.AxisListType.X, ALU.add)

            vcol = cp.tile([128, 4], f32, tag="vcol")
            nc.vector.tensor_copy(vcol[:, 0:1], partials[:, NEG:NEG + 1])
            nc.vector.tensor_reduce(vcol[:, 1:2], partials[:, 0:NEG],
                                    mybir.AxisListType.X, ALU.add)
            nc.vector.tensor_copy(vcol[:, 2:3], partials[:, NBLK:NBLK + 1])
            nc.any.memset(vcol[:, 3:4], 0.0)
            ones = cp.tile([128, 1], f32, tag="ones")
            nc.any.memset(ones[:], 1.0)
            psf = pp.tile([128, 2, 512], f32, tag="ps")
            nc.tensor.matmul(psf[0:1, 0, 0:4], ones[:], vcol[:],
                             start=True, stop=True)
            out_sb = cp.tile([1, 4], f32, tag="out_sb")
            nc.scalar.activation(out_sb[:], psf[0:1, 0, 0:4], AF.Copy)
            nc.sync.dma_start(out=d_out[:], in_=out_sb[:])

    nc.finalize()
    return nc


def _get_program(u_list, k_eff, b3f):
    key = (tuple(u_list), k_eff, float(b3f))
    if key not in _PROGRAM_CACHE:
        _PROGRAM_CACHE[key] = _build_program(u_list, k_eff, b3f)
    return _PROGRAM_CACHE[key]


# ------------------------------------------------------------------ kernel

def kernel(**inputs):
    u_list = [int(x) for x in np.asarray(inputs["unroll_subsample"]).reshape(-1)]
    k_eff = max(u_list) + 1
    w = _prep_weights(inputs)
    nc = _get_program(u_list, k_eff, w["b3f"])

    wmaps = {k: v for k, v in w.items() if k != "b3f"}
    in_maps = []
    for c in range(NC):
        m = dict(wmaps)
        m.update(_prep_core(c, inputs, u_list, k_eff))
        in_maps.append(m)

    res = bass_utils.run_bass_kernel_spmd(nc, in_maps, list(range(NC)))
    P = Ng = D = 0.0
    for c in range(NC):
        o = np.asarray(res.results[c]["out"], np.float64)
        P += o[0, 0]
        Ng += o[0, 1]
        D += o[0, 2]
    loss = COEFF * (P / D + Ng / (D * NEG))
    return np.float32(loss)


# revision 17
# speedup vs baseline: 1.6111x; 1.0120x over previous
"""Trainium2 Bass kernel for the CPCA auxiliary loss (nn_CPCA_51754355917033).

Strategy (data-parallel over the env/batch dim n, 16 envs per core):
  - GRU runs fully in bf16.  The input-side gate terms (x@W_ih.T + biases)
    are folded into the same PSUM accumulation as W_hh@h via one-hot action
    matmuls against a 19-row table (emb@W_ih.T + b_ih [+ b_hh]), so the only
    per-step vector work is the g-gate tail; r/z evict straight from PSUM
    through the scalar engine's sigmoid.  The gate tail is split into four
    128-row H-chunks so the next step's matmuls (which contract one H-chunk
    each) start as soon as the first chunk of h is ready.
  - MLP: preds@W1a + b1 is hoisted out of the 21-block loop (computed once,
    stashed as bf16 "p1e"); per block only x@W1b runs on the tensor engine
    and the eviction adds p1e (vector) + ReLU (scalar/vector split).
  - Host combines the 8 cores' (pos_sum, neg_sum, denom) partials into the
    final scalar (the all-reduce of the sharding hint, done at unshard time).
"""

import numpy as np
import ml_dtypes

import concourse.bass as bass
import concourse.mybir as mybir
import concourse.tile as tile
from concourse import bacc
from concourse import bass_utils

BF16 = ml_dtypes.bfloat16
F8 = ml_dtypes.float8_e4m3
DT = mybir.dt
AF = mybir.ActivationFunctionType
ALU = mybir.AluOpType

N, T, H, K, S, F, EMB, NLOG, NEG = 128, 512, 512, 16, 16, 4, 32, 18, 20
COEFF = 0.1
NC = 8
NPC = N // NC          # envs per core
R = NPC * S            # GRU rows per core (256)
L = T - 1
NBLK = NEG + 1         # 20 negative g-blocks + 1 positive block
BR = F * R             # rows per block (1024)
NA = NLOG + 1          # action vocab + padding row

_PROGRAM_CACHE = {}


# ----------------------------------------------------------------- host prep

def _prep_core(c, inputs, u_list, k_eff):
    acts = np.asarray(inputs["actions"])[..., 0]
    nd = np.asarray(inputs["not_dones"])[..., 0]
    ri = np.asarray(inputs["rnn_inputs"], np.float32)
    ro = np.asarray(inputs["rnn_outputs"], np.float32)
    ti = np.asarray(inputs["time_subsample"]).astype(np.int64)
    neg_idx = np.asarray(inputs["neg_idx"]).astype(np.int64)

    ns = slice(c * NPC, (c + 1) * NPC)
    idx = np.arange(k_eff)[:, None] + ti[None, :]          # (k_eff, S)

    act_ext = np.full((NPC, L + K), NLOG, np.int64)
    act_ext[:, :L] = acts[ns, :L]
    AI = act_ext[:, idx].transpose(1, 0, 2).reshape(k_eff, R)  # (k_eff, R)

    # one-hot actions padded to 128 partitions: oh[a, k, r] = (AI[k, r] == a)
    oh = np.zeros((128, k_eff, R), np.float32)
    kk = np.repeat(np.arange(k_eff), R)
    rr = np.tile(np.arange(R), k_eff)
    oh[AI.reshape(-1), kk, rr] = 1.0
    ohT = oh.astype(BF16)

    # g-gate input-side term, gathered on host: x@W_ih_g.T + b_ih_g
    W_ih = np.asarray(inputs["W_ih"], np.float32)
    b_ih = np.asarray(inputs["b_ih"], np.float32)
    emb_tab = np.asarray(inputs["action_embed"], np.float32)
    GIE_G = np.zeros((NA, 512), np.float32)
    GIE_G[:NLOG] = emb_tab @ W_ih[1024:].T + b_ih[1024:]
    GIE_G[NLOG] = b_ih[1024:]
    gig = GIE_G[AI]                                        # (k_eff, R, 512)
    gigT = np.ascontiguousarray(
        gig.transpose(0, 2, 1).reshape(k_eff, 4, 128, R)).astype(BF16)

    H0 = ro[ns][:, ti]                                     # (NPC, S, H)
    h0T = np.ascontiguousarray(
        H0.transpose(2, 0, 1).reshape(4, 128, R)).astype(BF16)

    ri_ext = np.zeros((NPC, L + K, H), np.float32)
    ri_ext[:, :L] = ri[ns, 1:]
    idx2 = np.asarray(u_list)[:, None] + ti[None, :]       # (F, S)
    TG = ri_ext[:, idx2]                                   # (NPC, F, S, H)
    tgT = np.ascontiguousarray(
        TG.transpose(3, 1, 0, 2).reshape(H, BR).reshape(4, 128, BR)).astype(F8)

    ni = neg_idx.reshape(F, N, S, NEG)[:, ns]              # (F, NPC, S, NEG)
    P = ni.transpose(3, 0, 1, 2).reshape(-1)               # cols in (g, f, j) order
    negs = ri.reshape(N * T, H)[P]
    negsT = np.ascontiguousarray(negs.T.reshape(4, 128, NEG * BR)).astype(F8)

    nd_ext = np.zeros((NPC, L + K), np.float32)
    nd_ext[:, :L] = nd[ns, :L]
    G = nd_ext[:, idx]                                     # (NPC, k_eff, S)
    ndv = G.transpose(1, 0, 2).reshape(k_eff, R)
    ndvT = np.ascontiguousarray(
        ndv.reshape(k_eff, 2, 128).transpose(2, 0, 1)).astype(np.float32)

    return dict(ohT=ohT, gigT=gigT, h0T=h0T, tgT=tgT, negsT=negsT, ndvT=ndvT)


def _prep_weights(inputs):
    W_ih = np.asarray(inputs["W_ih"], np.float32)
    W_hh = np.asarray(inputs["W_hh"], np.float32)
    b_ih = np.asarray(inputs["b_ih"], np.float32)
    b_hh = np.asarray(inputs["b_hh"], np.float32)
    emb_tab = np.asarray(inputs["action_embed"], np.float32)
    W1 = np.asarray(inputs["W1"], np.float32)
    b1 = np.asarray(inputs["b1"], np.float32)
    W2 = np.asarray(inputs["W2"], np.float32)
    b2 = np.asarray(inputs["b2"], np.float32)
    W3 = np.asarray(inputs["W3"], np.float32)
    b3 = np.asarray(inputs["b3"], np.float32)

    d = {}
    # W_hh.T packed for fp8 DoubleRow: [th, ki, ko, 1536]
    d["whh8"] = np.ascontiguousarray(
        W_hh.T.reshape(2, 2, 128, 1536).transpose(0, 2, 1, 3)).astype(F8)
    # gie: action table, padded to 128 rows and chunked contiguously
    # ([128, 12, 128] so each stationary slice is contiguous -> FWL).
    # cols 0:1024 = emb@W_ih_rz.T + b_ih_rz + b_hh_rz (complete r/z
    # input-side term); cols 1024:1536 = b_hh_g broadcast (all rows
    # identical -> the one-hot matmul adds b_hh_g to the g psum).
    gie = np.zeros((128, 1536), np.float32)
    gie[:NLOG, :1024] = emb_tab @ W_ih[:1024].T + b_ih[:1024] + b_hh[:1024]
    gie[NLOG, :1024] = b_ih[:1024] + b_hh[:1024]
    gie[:NA, 1024:] = b_hh[1024:][None, :]
    d["gie"] = np.ascontiguousarray(
        gie.reshape(128, 12, 128)).astype(BF16)

    def pack8(WT):
        # [t, ki, ko, m] with contract index = t*256 + ko*128 + ki
        return np.ascontiguousarray(
            WT.reshape(2, 2, 128, WT.shape[1]).transpose(0, 2, 1, 3)).astype(F8)
    d["w1a8"] = pack8(W1[:, :512].T.copy())
    d["w1b8"] = pack8(W1[:, 512:].T.copy())
    d["w28"] = pack8(W2.T.copy())
    d["w3T"] = np.ascontiguousarray(W3[0].reshape(4, 128).T).astype(BF16)
    d["b1T"] = np.ascontiguousarray(b1.reshape(4, 128).T).astype(np.float32)
    d["b2T"] = np.ascontiguousarray(b2.reshape(4, 128).T).astype(np.float32)
    d["b3f"] = float(b3.reshape(-1)[0])
    return d


# ------------------------------------------------------------- device program

def _gsl(t, c):
    """[128, 2, 512] tile viewed as 4 chunks of [128, 256]."""
    return t[:, c // 2, (c % 2) * 256:(c % 2) * 256 + 256]


def _build_program(u_list, k_eff, b3f):
    nc = bacc.Bacc("TRN2", target_bir_lowering=False, debug=False, num_devices=NC)

    di = {}
    def inp(name, shape, dt):
        di[name] = nc.dram_tensor(name, list(shape), dt, kind="ExternalInput")
        return di[name]

    d_whh = inp("whh8", (2, 128, 2, 1536), DT.float8e4)
    d_gie = inp("gie", (128, 12, 128), DT.bfloat16)
    d_oh = inp("ohT", (128, k_eff, R), DT.bfloat16)
    d_gig = inp("gigT", (k_eff, 4, 128, R), DT.bfloat16)
    d_w1a = inp("w1a8", (2, 128, 2, 512), DT.float8e4)
    d_w1b = inp("w1b8", (2, 128, 2, 512), DT.float8e4)
    d_w2 = inp("w28", (2, 128, 2, 512), DT.float8e4)
    d_w3 = inp("w3T", (128, 4), DT.bfloat16)
    d_b1 = inp("b1T", (128, 4), DT.float32)
    d_b2 = inp("b2T", (128, 4), DT.float32)
    d_h0 = inp("h0T", (4, 128, R), DT.bfloat16)
    d_tg = inp("tgT", (4, 128, BR), DT.float8e4)
    d_negs = inp("negsT", (4, 128, NEG * BR), DT.float8e4)
    d_ndv = inp("ndvT", (128, k_eff, 2), DT.float32)
    d_out = nc.dram_tensor("out", [1, 4], DT.float32, kind="ExternalOutput")

    f32 = DT.float32
    bf16 = DT.bfloat16
    f8 = DT.float8e4

    with tile.TileContext(nc) as tc:
        with (
            tc.tile_pool(name="const", bufs=1) as cp,
            tc.tile_pool(name="gruw", bufs=2) as gp,
            tc.tile_pool(name="mlpw", bufs=3) as mp,
            tc.tile_pool(name="ps", bufs=4, space="PSUM") as pp,
        ):
            # ------------------------------------------------ constant loads
            whh = cp.tile([128, 2, 2, 1536], f8, tag="whh")
            for th in range(2):
                nc.sync.dma_start(out=whh[:, th, :, :], in_=d_whh[th])
            gie = cp.tile([128, 12, 128], bf16, tag="gie")
            nc.sync.dma_start(out=gie[:], in_=d_gie[:])
            oh = cp.tile([128, k_eff, R], bf16, tag="oh")
            nc.sync.dma_start(out=oh[:], in_=d_oh[:])
            w1a = cp.tile([128, 2, 2, 512], f8, tag="w1a")
            w1b = cp.tile([128, 2, 2, 512], f8, tag="w1b")
            w2 = cp.tile([128, 2, 2, 512], f8, tag="w2")
            for (t, d) in ((w1a, d_w1a), (w1b, d_w1b), (w2, d_w2)):
                for th in range(2):
                    nc.sync.dma_start(out=t[:, th, :, :], in_=d[th])
            w3 = cp.tile([128, 4], bf16, tag="w3")
            nc.sync.dma_start(out=w3[:], in_=d_w3[:])
            b1 = cp.tile([128, 4], f32, tag="b1")
            nc.sync.dma_start(out=b1[:], in_=d_b1[:])
            b2 = cp.tile([128, 4], f32, tag="b2")
            nc.sync.dma_start(out=b2[:], in_=d_b2[:])
            tg = cp.tile([128, 4, BR], f8, tag="tg")
            for kc in range(4):
                nc.sync.dma_start(out=tg[:, kc, :], in_=d_tg[kc])
            ndv = cp.tile([128, k_eff, 2], f32, tag="ndv")
            nc.sync.dma_start(out=ndv[:], in_=d_ndv[:])

            # ------------------------------------------------ forward mask
            prod = cp.tile([128, k_eff, 2], f32, tag="prod")
            nc.vector.tensor_scalar(prod[:, 0, :], ndv[:, 0, :], 0.0, None,
                                    op0=ALU.is_gt)
            for k in range(1, k_eff):
                nc.vector.scalar_tensor_tensor(
                    prod[:, k, :], in0=ndv[:, k, :], scalar=0.0,
                    in1=prod[:, k - 1, :], op0=ALU.is_gt, op1=ALU.mult)
            mfT = cp.tile([128, 2 * F], f32, tag="mfT")
            for fi, u in enumerate(u_list):
                nc.vector.tensor_copy(mfT[:, 2 * fi:2 * fi + 2], prod[:, u, :])

            # ------------------------------------------------ GRU (bf16)
            # Gate order in the 1536 dim: r(512), z(512), g(512).
            # Per step: 48 W_hh matmuls + 12 one-hot matmuls (gi / biases
            # folded into PSUM).  r/z sigmoid straight from PSUM on the
            # scalar engine; g tail per 128-H-chunk so the next step's
            # matmuls start as soon as chunk 0 of h_new is written.
            h_prev = gp.tile([128, 4, R], bf16, tag="h")
            for kc in range(4):
                nc.sync.dma_start(out=h_prev[:, kc, :], in_=d_h0[kc])
            h8_prev = gp.tile([128, 4, R], f8, tag="h8")
            nc.vector.tensor_copy(h8_prev[:], h_prev[:])
            predsT = cp.tile([128, 4, BR], f8, tag="preds")
            p1e = cp.tile([128, 4, BR], bf16, tag="p1e")
            DRM = mybir.MatmulPerfMode.DoubleRow

            def p1_chunk(fi):
                # p1e[:, :, fi*R:] = preds_fi @ W1a + b1 (hoisted L1 half)
                rsl = slice(fi * R, (fi + 1) * R)
                ps = pp.tile([128, 2, 512], f32, tag="ps")
                for cc in range(4):
                    for th in range(2):
                        nc.tensor.matmul(
                            _gsl(ps, cc),
                            w1a[:, th, :, cc * 128:(cc + 1) * 128],
                            predsT[:, 2 * th:2 * th + 2, rsl],
                            start=(th == 0), stop=(th == 1), perf_mode=DRM)
                for cc in range(4):
                    nc.scalar.activation(p1e[:, cc, rsl], _gsl(ps, cc),
                                         AF.Identity, bias=b1[:, cc:cc + 1])

            for k in range(k_eff):
                gig = gp.tile([128, 4, R], bf16, tag="gig", bufs=3)
                for kc in range(4):
                    nc.sync.dma_start(out=gig[:, kc, :], in_=d_gig[k, kc])

                ps_r = pp.tile([128, 2, 512], f32, tag="ps")
                ps_g = pp.tile([128, 2, 512], f32, tag="ps")
                ps_z = pp.tile([128, 2, 512], f32, tag="ps")
                r_sb = gp.tile([128, 4, R], bf16, tag="r")
                z_sb = gp.tile([128, 4, R], bf16, tag="z")
                t_sb = gp.tile([128, 4, R], bf16, tag="t")
                u_sb = gp.tile([128, 4, R], bf16, tag="u")
                g_sb = gp.tile([128, 4, R], bf16, tag="g")
                d_sb = gp.tile([128, 4, R], bf16, tag="d")
                hz_sb = gp.tile([128, 4, R], bf16, tag="hz")
                h_new = gp.tile([128, 4, R], bf16, tag="h")
                h8_new = gp.tile([128, 4, R], f8, tag="h8")

                # psum region for gate chunk: r = ps_r c0..3, z = ps_z,
                # g = ps_g.  MM emission is 3-phase so the tensor queue never
                # stalls: one-hot matmuls first (no h dependency -> they run
                # during the previous step's tail), then all th=0 DR matmuls
                # (need only h8 pair 0), then th=1 ordered r, g, z.
                def reg(gc):
                    return _gsl((ps_r, ps_z, ps_g)[gc // 4], gc % 4)

                for gc in range(12):
                    nc.tensor.matmul(reg(gc), gie[:, gc, :], oh[:, k, :],
                                     start=True, stop=False)
                for order in ((0, 1, 2), (0, 2, 1)):
                    th = 0 if order == (0, 1, 2) else 1
                    for grp in order:
                        for c in range(4):
                            gc = grp * 4 + c
                            sl = slice(gc * 128, (gc + 1) * 128)
                            nc.tensor.matmul(
                                reg(gc), whh[:, th, :, sl],
                                h8_prev[:, 2 * th:2 * th + 2, :],
                                start=False, stop=(th == 1), perf_mode=DRM)
                # tail in half-batches (chunk pairs).  The f8 h8_new pair
                # is written directly by the final add, so the next step's
                # th=0 matmuls start as soon as pair 0 lands; the bf16 copy
                # of h (needed for the next step's d = h - g) trails lazily.
                def half(tl, p):
                    return tl[:, 2 * p:2 * p + 2, :]
                def tail(p):
                    nc.scalar.activation(half(r_sb, p), ps_r[:, p, :],
                                         AF.Sigmoid)
                    nc.vector.tensor_mul(half(t_sb, p), ps_g[:, p, :],
                                         half(r_sb, p))
                    nc.vector.tensor_add(half(u_sb, p), half(gig, p),
                                         half(t_sb, p))
                    nc.scalar.activation(half(z_sb, p), ps_z[:, p, :],
                                         AF.Sigmoid)
                    nc.scalar.activation(half(g_sb, p), half(u_sb, p),
                                         AF.Tanh)
                    nc.vector.tensor_sub(half(d_sb, p), half(h_prev, p),
                                         half(g_sb, p))
                    nc.vector.tensor_mul(half(hz_sb, p), half(z_sb, p),
                                         half(d_sb, p))
                    nc.vector.tensor_add(half(h8_new, p), half(g_sb, p),
                                         half(hz_sb, p))
                tail(0)
                tail(1)
                nc.vector.tensor_add(h_new[:, 0:2, :], g_sb[:, 0:2, :],
                                     hz_sb[:, 0:2, :])
                nc.vector.tensor_add(h_new[:, 2:4, :], g_sb[:, 2:4, :],
                                     hz_sb[:, 2:4, :])
                h_prev = h_new
                h8_prev = h8_new
                for fi, u in enumerate(u_list):
                    if u == k:
                        nc.vector.tensor_copy(
                            predsT[:, :, fi * R:(fi + 1) * R], h_new[:])

            for fi in range(F):
                p1_chunk(fi)

            DR = mybir.MatmulPerfMode.DoubleRow

            # ------------------------------------------------ blocks
            # Software-pipelined: iteration b emits L1(b), L2(b-1), L3(b-2).
            # Block 0 is the POSITIVE block (targets, preloaded) so its
            # softplus tail runs early; blocks 1..20 are the negatives.
            # Masked softplus via fold: softplus((logit + BIG + b3)*mf - BIG)
            # equals softplus(logit + b3) for mf=1 and ~0 for mf=0, so the
            # whole masked sum is one activation with a fused accum_out.
            BIG = 30.0
            logits = cp.tile([128, NBLK, 8], f32, tag="logits")
            nlm = cp.tile([128, NEG, 8], f32, tag="nlm")
            plm = cp.tile([128, 8], f32, tag="plm")
            sp_n = cp.tile([128, NEG, 8], f32, tag="sp_n")
            sp_na = cp.tile([128, NEG, 8], f32, tag="sp_na")
            sp_p = cp.tile([128, 8], f32, tag="sp_p")
            sp_pa = cp.tile([128, 8], f32, tag="sp_pa")
            vcol = cp.tile([128, 4], f32, tag="vcol")
            bigm = cp.tile([128, 1], f32, tag="bigm")
            nc.any.memset(bigm[:], -BIG)
            nc.vector.tensor_reduce(vcol[:, 2:3], mfT[:],
                                    mybir.AxisListType.X, ALU.add)
            nc.any.memset(vcol[:, 3:4], 0.0)
            l2_pend = []
            l3_pend = []
            for b in range(NBLK + 2):
                if b < NBLK:
                    if b > 0:
                        q = b - 1
                        xt = mp.tile([128, 4, BR], f8, tag="negsx")
                        for kc in range(4):
                            nc.sync.dma_start(
                                out=xt[:, kc, :],
                                in_=d_negs[kc][:, q * BR:(q + 1) * BR])
                    else:
                        xt = tg
                    y1 = mp.tile([128, 4, BR], f8, tag="y1", bufs=2)
                    y1p = mp.tile([128, 4, BR], bf16, tag="y1p", bufs=2)
                    for cc in range(4):
                        ps = pp.tile([128, 2, 512], f32, tag="ps")
                        for rt in range(2):
                            sl = slice(rt * 512, (rt + 1) * 512)
                            for th in range(2):
                                nc.tensor.matmul(
                                    ps[:, rt, :],
                                    w1b[:, th, :, cc * 128:(cc + 1) * 128],
                                    xt[:, 2 * th:2 * th + 2, sl],
                                    start=(th == 0), stop=(th == 1),
                                    perf_mode=DR)
                        nc.vector.scalar_tensor_tensor(
                            y1p[:, cc, :], in0=ps[:], scalar=0.0,
                            in1=p1e[:, cc, :], op0=ALU.add, op1=ALU.add)
                        if cc < 2:
                            nc.scalar.activation(y1[:, cc, :], y1p[:, cc, :],
                                                 AF.Relu)
                        else:
                            nc.vector.tensor_scalar(y1[:, cc, :],
                                                    y1p[:, cc, :],
                                                    0.0, None, op0=ALU.max)
                    l2_pend.append(y1)
                if b >= 1 and l2_pend:
                    yy1 = l2_pend.pop(0)
                    y2 = mp.tile([128, 4, BR], bf16, tag="y2", bufs=2)
                    for cc in range(4):
                        ps = pp.tile([128, 2, 512], f32, tag="ps")
                        for rt in range(2):
                            sl = slice(rt * 512, (rt + 1) * 512)
                            for th in range(2):
                                nc.tensor.matmul(
                                    ps[:, rt, :],
                                    w2[:, th, :, cc * 128:(cc + 1) * 128],
                                    yy1[:, 2 * th:2 * th + 2, sl],
                                    start=(th == 0), stop=(th == 1),
                                    perf_mode=DR)
                        nc.scalar.activation(y2[:, cc, :], ps[:], AF.Relu,
                                             bias=b2[:, cc:cc + 1])
                    l3_pend.append(y2)
                if b >= 2 and l3_pend:
                    bb = b - 2
                    yy2 = l3_pend.pop(0)
                    ps3 = pp.tile([128, 2, 512], f32, tag="ps")
                    for col in range(8):
                        for kc in range(4):
                            nc.tensor.matmul(
                                ps3[:, 0, col:col + 1],
                                yy2[:, kc, col * 128:(col + 1) * 128],
                                w3[:, kc:kc + 1], start=(kc == 0),
                                stop=(kc == 3))
                    nc.scalar.activation(logits[:, bb, :], ps3[:, 0, 0:8],
                                         AF.Copy)
                    if bb == 0:
                        nc.vector.scalar_tensor_tensor(
                            plm[:], in0=logits[:, 0, :], scalar=BIG - b3f,
                            in1=mfT[:], op0=ALU.subtract, op1=ALU.mult)
                        # softplus(x) = relu(x) - ln(sigmoid(|x|)), x = -plm-BIG
                        nc.scalar.activation(sp_pa[:], plm[:], AF.Abs,
                                             bias=bigm[:], scale=-1.0)
                        nc.scalar.activation(sp_pa[:], sp_pa[:], AF.Sigmoid)
                        nc.scalar.activation(sp_pa[:], sp_pa[:], AF.Ln)
                        nc.scalar.activation(sp_p[:], plm[:], AF.Relu,
                                             bias=bigm[:], scale=-1.0)
                        nc.vector.scalar_tensor_tensor(
                            sp_p[:], in0=sp_p[:], scalar=0.0, in1=sp_pa[:],
                            op0=ALU.add, op1=ALU.subtract,
                            accum_out=vcol[:, 0:1])
                    else:
                        nc.vector.scalar_tensor_tensor(
                            nlm[:, bb - 1, :], in0=logits[:, bb, :],
                            scalar=BIG + b3f, in1=mfT[:],
                            op0=ALU.add, op1=ALU.mult)
            nc.scalar.activation(sp_na[:], nlm[:], AF.Abs, bias=bigm[:])
            nc.scalar.activation(sp_na[:], sp_na[:], AF.Sigmoid)
            nc.scalar.activation(sp_na[:], sp_na[:], AF.Ln)
            nc.scalar.activation(sp_n[:], nlm[:], AF.Relu, bias=bigm[:])
            nc.vector.scalar_tensor_tensor(
                sp_n[:], in0=sp_n[:], scalar=0.0, in1=sp_na[:],
                op0=ALU.add, op1=ALU.subtract, accum_out=vcol[:, 1:2])

            ones = cp.tile([128, 1], f32, tag="ones")
            nc.any.memset(ones[:], 1.0)
            psf = pp.tile([128, 2, 512], f32, tag="ps")
            nc.tensor.matmul(psf[0:1, 0, 0:4], ones[:], vcol[:],
                             start=True, stop=True)
            out_sb = cp.tile([1, 4], f32, tag="out_sb")
            nc.scalar.activation(out_sb[:], psf[0:1, 0, 0:4], AF.Copy)
            nc.sync.dma_start(out=d_out[:], in_=out_sb[:])

    nc.finalize()
    return nc


def _get_program(u_list, k_eff, b3f):
    key = (tuple(u_list), k_eff, float(b3f))
    if key not in _PROGRAM_CACHE:
        _PROGRAM_CACHE[key] = _build_program(u_list, k_eff, b3f)
    return _PROGRAM_CACHE[key]


# ------------------------------------------------------------------ kernel

def kernel(**inputs):
    u_list = [int(x) for x in np.asarray(inputs["unroll_subsample"]).reshape(-1)]
    k_eff = max(u_list) + 1
    w = _prep_weights(inputs)
    nc = _get_program(u_list, k_eff, w["b3f"])

    wmaps = {k: v for k, v in w.items() if k != "b3f"}
    in_maps = []
    for c in range(NC):
        m = dict(wmaps)
        m.update(_prep_core(c, inputs, u_list, k_eff))
        in_maps.append(m)

    res = bass_utils.run_bass_kernel_spmd(nc, in_maps, list(range(NC)))
    P = Ng = D = 0.0
    for c in range(NC):
        o = np.asarray(res.results[c]["out"], np.float64)
        P += o[0, 0]
        Ng += o[0, 1]
        D += o[0, 2]
    loss = COEFF * (P / D + Ng / (D * NEG))
    return np.float32(loss)
